# revision 1
# baseline (speedup 1.0000x reference)
"""DGCNN prediction head on 8 Trainium2 NeuronCores.

Data-parallel over batch B=8: each core runs the full pipeline for one
sample (C=64 channels, N=4096 points, k=20 neighbors).

Per-core pipeline (all on one NeuronCore, no collectives):
  1. pairwise ranking R[i,j] = 2<x_i,x_j> - ||x_j||^2 via PE matmul with an
     augmented contract row (row 64 of lhsT = -1, row 64 of rhs = ||x_j||^2).
     (-||x_i||^2 is a per-row constant and cannot change the top-k order.)
  2. exact top-20 per row with DVE max8/max_index/match_replace (3 rounds).
  3. EdgeConv1 is linear before the LReLU, so it is precomputed per point:
       conv1(i,j) = Wn x_j + (Wc - Wn) x_i  with BN1 folded in
     A' = s1*(Wn x)        -> transposed to DRAM table, row-gathered by index
     B' = s1*((Wc-Wn) x)+t1-> kept on-chip, broadcast-added per query block
  4. e1 = lrelu(A'_j + B'_i) per edge; PE-transpose to channel-major;
     EdgeConv2 as 64x64 matmul (BN2 scale folded into W2, bias t2 added
     during the PSUM drain); max over k on GPSIMD; lrelu (monotone ops
     commute with max since s2 >= 0).
  5. point MLP 64->256->128->1 with BN scales folded into weights, biases
     added during PSUM drains, lrelu on GPSIMD.
"""

import numpy as np

C = 64
K = 20
NEG = 0.2
EPS = 1e-5
NCORES = 8
N_FULL = 4096
NEG_FILL = -3.0e38

_cache = {}


def build_nc(n):
    from contextlib import ExitStack

    import concourse.bass as bass
    import concourse.bacc as bacc
    import concourse.mybir as mybir
    import concourse.tile as tile
    from concourse.masks import make_identity

    f32 = mybir.dt.float32
    u32 = mybir.dt.uint32
    AF = mybir.ActivationFunctionType
    OP = mybir.AluOpType

    nblk = n // 128
    nchk = n // 512

    nc = bacc.Bacc("TRN2", target_bir_lowering=False, debug=False,
                   num_devices=NCORES)

    x_d = nc.dram_tensor("x", [C, n], f32, kind="ExternalInput")
    wnT_d = nc.dram_tensor("wnT", [C, C], f32, kind="ExternalInput")
    wcnT_d = nc.dram_tensor("wcnT", [C, C], f32, kind="ExternalInput")
    t1_d = nc.dram_tensor("t1", [C, 1], f32, kind="ExternalInput")
    w2T_d = nc.dram_tensor("w2T", [C, C], f32, kind="ExternalInput")
    t2_d = nc.dram_tensor("t2", [C, 1], f32, kind="ExternalInput")
    w1aT_d = nc.dram_tensor("w1aT", [C, 128], f32, kind="ExternalInput")
    w1bT_d = nc.dram_tensor("w1bT", [C, 128], f32, kind="ExternalInput")
    tm1a_d = nc.dram_tensor("tm1a", [128, 1], f32, kind="ExternalInput")
    tm1b_d = nc.dram_tensor("tm1b", [128, 1], f32, kind="ExternalInput")
    w2maT_d = nc.dram_tensor("w2maT", [128, 128], f32, kind="ExternalInput")
    w2mbT_d = nc.dram_tensor("w2mbT", [128, 128], f32, kind="ExternalInput")
    tm2_d = nc.dram_tensor("tm2", [128, 1], f32, kind="ExternalInput")
    w3T_d = nc.dram_tensor("w3T", [128, 1], f32, kind="ExternalInput")
    b3_d = nc.dram_tensor("b3", [1, 1], f32, kind="ExternalInput")
    out_d = nc.dram_tensor("out", [1, n], f32, kind="ExternalOutput")

    with tile.TileContext(nc) as tc, ExitStack() as top:
        cpool = top.enter_context(tc.tile_pool(name="consts", bufs=1))
        dpool = top.enter_context(tc.tile_pool(name="dram", bufs=1, space="DRAM"))
        xpool = top.enter_context(tc.tile_pool(name="xaug", bufs=1))
        hpool = top.enter_context(tc.tile_pool(name="hout", bufs=1))

        # --- constants / weights ---
        ident = cpool.tile([128, 128], f32, tag="ident")
        make_identity(nc, ident[:])
        ones64 = cpool.tile([C, 1], f32, tag="ones64")
        nc.vector.memset(ones64[:], 1.0)

        def load_const(dram, shape, tag):
            t = cpool.tile(shape, f32, tag=tag)
            nc.sync.dma_start(t[:], dram[:])
            return t

        wnT = load_const(wnT_d, [C, C], "wnT")
        wcnT = load_const(wcnT_d, [C, C], "wcnT")
        t1 = load_const(t1_d, [C, 1], "t1")
        w2T = load_const(w2T_d, [C, C], "w2T")
        t2 = load_const(t2_d, [C, 1], "t2")
        w1aT = load_const(w1aT_d, [C, 128], "w1aT")
        w1bT = load_const(w1bT_d, [C, 128], "w1bT")
        tm1a = load_const(tm1a_d, [128, 1], "tm1a")
        tm1b = load_const(tm1b_d, [128, 1], "tm1b")
        w2maT = load_const(w2maT_d, [128, 128], "w2maT")
        w2mbT = load_const(w2mbT_d, [128, 128], "w2mbT")
        tm2 = load_const(tm2_d, [128, 1], "tm2")
        w3T = load_const(w3T_d, [128, 1], "w3T")
        b3 = load_const(b3_d, [1, 1], "b3")

        At = dpool.tile([n, C], f32, tag="At")          # A' transposed table
        xaug = xpool.tile([C + 1, n], f32, tag="xaug")   # rows 0..63 = x, row 64 = ||x_j||^2
        x2aug = xpool.tile([C + 1, n], f32, tag="x2aug") # rows 0..63 = 2x, row 64 = -1
        Bt = xpool.tile([128, C * nblk], f32, tag="Bt")  # B' transposed, block j at cols 64j
        H = hpool.tile([C, n], f32, tag="H")             # per-point features after edge max
        osb = hpool.tile([1, n], f32, tag="osb")

        # ---------------- stage 0: tables ----------------
        with tc.tile_pool(name="s0sb", bufs=2) as s0sb, \
             tc.tile_pool(name="s0ps", bufs=3, space="PSUM") as s0ps:
            nc.sync.dma_start(xaug[:C, :], x_d[:])
            nc.scalar.activation(out=x2aug[:C, :], in_=xaug[:C, :],
                                 func=AF.Copy, scale=2.0)
            nc.vector.memset(x2aug[C:C + 1, :], -1.0)
            for ch in range(nchk):
                cs = slice(512 * ch, 512 * (ch + 1))
                xsq = s0sb.tile([C, 512], f32, tag="xsq")
                nc.scalar.activation(out=xsq[:], in_=xaug[:C, cs], func=AF.Square)
                psxx = s0ps.tile([1, 512], f32, tag="s0p", space="PSUM")
                nc.tensor.matmul(out=psxx[:], lhsT=ones64[:], rhs=xsq[:],
                                 start=True, stop=True)
                nc.scalar.copy(out=xaug[C:C + 1, cs], in_=psxx[:])
            for ch in range(nchk):
                cs = slice(512 * ch, 512 * (ch + 1))
                psa = s0ps.tile([C, 512], f32, tag="s0p", space="PSUM")
                nc.tensor.matmul(out=psa[:], lhsT=wnT[:], rhs=xaug[:C, cs],
                                 start=True, stop=True)
                ap = s0sb.tile([C, 512], f32, tag="ap")
                nc.scalar.copy(out=ap[:], in_=psa[:])
                psb = s0ps.tile([C, 512], f32, tag="s0p", space="PSUM")
                nc.tensor.matmul(out=psb[:], lhsT=wcnT[:], rhs=xaug[:C, cs],
                                 start=True, stop=True)
                bp = s0sb.tile([C, 512], f32, tag="bp")
                nc.scalar.activation(out=bp[:], in_=psb[:], func=AF.Identity,
                                     bias=t1[:], scale=1.0)
                for j in range(4):
                    blk = 4 * ch + j
                    js = slice(128 * j, 128 * (j + 1))
                    pta = s0ps.tile([128, C], f32, tag="s0p", space="PSUM")
                    nc.tensor.transpose(out=pta[:], in_=ap[:, js],
                                        identity=ident[:C, :C])
                    ast = s0sb.tile([128, C], f32, tag="ast")
                    nc.scalar.copy(out=ast[:], in_=pta[:])
                    nc.sync.dma_start(At[128 * blk:128 * (blk + 1), :], ast[:])
                    ptb = s0ps.tile([128, C], f32, tag="s0p", space="PSUM")
                    nc.tensor.transpose(out=ptb[:], in_=bp[:, js],
                                        identity=ident[:C, :C])
                    nc.scalar.copy(out=Bt[:, C * blk:C * (blk + 1)], in_=ptb[:])

        # ---------------- stage 1: blocks ----------------
        with tc.tile_pool(name="rpool", bufs=2) as rpool, \
             tc.tile_pool(name="vpool", bufs=8) as vpool, \
             tc.tile_pool(name="gpool", bufs=2) as gpool, \
             tc.tile_pool(name="epool", bufs=2) as epool, \
             tc.tile_pool(name="wpool", bufs=2) as wpool, \
             tc.tile_pool(name="tpool", bufs=2) as tpool, \
             tc.tile_pool(name="psR", bufs=2, space="PSUM") as psR, \
             tc.tile_pool(name="psT", bufs=2, space="PSUM") as psT, \
             tc.tile_pool(name="psE", bufs=2, space="PSUM") as psE:

            r_tiles = {}

            def emit_pairwise(b):
                R0 = rpool.tile([128, n], f32, tag="R")
                bs = slice(128 * b, 128 * (b + 1))
                for ch in range(nchk):
                    cs = slice(512 * ch, 512 * (ch + 1))
                    ps = psR.tile([128, 512], f32, tag="psr", space="PSUM")
                    nc.tensor.matmul(out=ps[:], lhsT=x2aug[:, bs],
                                     rhs=xaug[:, cs], start=True, stop=True)
                    nc.scalar.copy(out=R0[:, cs], in_=ps[:])
                r_tiles[b] = R0

            def emit_edge(b):
                R0 = r_tiles.pop(b)
                bs = slice(128 * b, 128 * (b + 1))
                v1 = vpool.tile([128, 8], f32, tag="v1")
                v2 = vpool.tile([128, 8], f32, tag="v2")
                v3 = vpool.tile([128, 8], f32, tag="v3")
                i1 = vpool.tile([128, 8], u32, tag="i1")
                i2 = vpool.tile([128, 8], u32, tag="i2")
                i3 = vpool.tile([128, 8], u32, tag="i3")
                nc.vector.max(out=v1[:], in_=R0[:])
                nc.vector.max_index(out=i1[:], in_max=v1[:], in_values=R0[:])
                nc.vector.match_replace(out=R0[:], in_to_replace=v1[:],
                                        in_values=R0[:], imm_value=NEG_FILL)
                nc.vector.max(out=v2[:], in_=R0[:])
                nc.vector.max_index(out=i2[:], in_max=v2[:], in_values=R0[:])
                nc.vector.match_replace(out=R0[:], in_to_replace=v2[:],
                                        in_values=R0[:], imm_value=NEG_FILL)
                nc.vector.max(out=v3[:], in_=R0[:])
                nc.vector.max_index(out=i3[:], in_max=v3[:], in_values=R0[:])

                G = gpool.tile([128, K * C], f32, tag="G")
                isrc = [i1] * 8 + [i2] * 8 + [i3] * 4
                for k in range(K):
                    col = k % 8
                    nc.gpsimd.indirect_dma_start(
                        out=G[:, C * k:C * (k + 1)], out_offset=None,
                        in_=At[:],
                        in_offset=bass.IndirectOffsetOnAxis(
                            ap=isrc[k][:, col:col + 1], axis=0))

                # e1 = lrelu(G + B'_i)
                bb = Bt[:, C * b:C * (b + 1)].rearrange(
                    "p (k c) -> p k c", k=1).to_broadcast([128, K, C])
                nc.vector.tensor_tensor(
                    out=G[:].rearrange("p (k c) -> p k c", k=K),
                    in0=G[:].rearrange("p (k c) -> p k c", k=K),
                    in1=bb, op=OP.add)
                nc.vector.scalar_tensor_tensor(
                    out=G[:], in0=G[:], scalar=NEG, in1=G[:],
                    op0=OP.mult, op1=OP.max)

                # transpose to channel-major: 20 PE transposes [128,64]->[64,128]
                e1T = gpool.tile([C, K * 128], f32, tag="e1T")
                for grp in range(5):
                    pt = psT.tile([C, 512], f32, tag="pst", space="PSUM")
                    for s in range(4):
                        k = 4 * grp + s
                        nc.tensor.transpose(
                            out=pt[:, 128 * s:128 * (s + 1)],
                            in_=G[:, C * k:C * (k + 1)],
                            identity=ident[:])
                    nc.scalar.copy(out=e1T[:, 512 * grp:512 * (grp + 1)],
                                   in_=pt[:])

                # conv2 (w_k2 with bn2 scale folded), t2 added in drain
                ew = wpool.tile([C, K * 128], f32, tag="ew")
                for grp in range(5):
                    pe = psE.tile([C, 512], f32, tag="pse", space="PSUM")
                    for s in range(4):
                        k = 4 * grp + s
                        nc.tensor.matmul(
                            out=pe[:, 128 * s:128 * (s + 1)],
                            lhsT=w2T[:],
                            rhs=e1T[:, 128 * k:128 * (k + 1)],
                            start=True, stop=True)
                    nc.scalar.activation(
                        out=ew[:, 512 * grp:512 * (grp + 1)], in_=pe[:],
                        func=AF.Identity, bias=t2[:], scale=1.0)

                # max over k (GPSIMD tree), then lrelu -> H
                m1 = tpool.tile([C, 10 * 128], f32, tag="m1")
                nc.vector.tensor_tensor(out=m1[:], in0=ew[:, :1280],
                                        in1=ew[:, 1280:], op=OP.max)
                m2 = tpool.tile([C, 5 * 128], f32, tag="m2")
                nc.vector.tensor_tensor(out=m2[:], in0=m1[:, :640],
                                        in1=m1[:, 640:], op=OP.max)
                m3 = tpool.tile([C, 2 * 128], f32, tag="m3")
                nc.vector.tensor_tensor(out=m3[:], in0=m2[:, :256],
                                        in1=m2[:, 256:512], op=OP.max)
                m4 = tpool.tile([C, 128], f32, tag="m4")
                nc.vector.tensor_tensor(out=m4[:], in0=m3[:, :128],
                                        in1=m3[:, 128:], op=OP.max)
                nc.vector.tensor_tensor(out=m4[:], in0=m4[:],
                                        in1=m2[:, 512:], op=OP.max)
                nc.vector.scalar_tensor_tensor(
                    out=H[:, bs], in0=m4[:], scalar=NEG, in1=m4[:],
                    op0=OP.mult, op1=OP.max)

            emit_pairwise(0)
            for b in range(nblk):
                if b + 1 < nblk:
                    emit_pairwise(b + 1)
                emit_edge(b)

        # ---------------- stage 2: point MLP ----------------
        with tc.tile_pool(name="mlpsb", bufs=2) as mlpsb, \
             tc.tile_pool(name="mlpps", bufs=4, space="PSUM") as mlpps:
            for ch in range(nchk):
                cs = slice(512 * ch, 512 * (ch + 1))
                l1a = mlpsb.tile([128, 512], f32, tag="l1a")
                l1b = mlpsb.tile([128, 512], f32, tag="l1b")
                ps1a = mlpps.tile([128, 512], f32, tag="mlpp", space="PSUM")
                nc.tensor.matmul(out=ps1a[:], lhsT=w1aT[:], rhs=H[:, cs],
                                 start=True, stop=True)
                nc.scalar.activation(out=l1a[:], in_=ps1a[:],
                                     func=AF.Identity, bias=tm1a[:], scale=1.0)
                nc.vector.scalar_tensor_tensor(
                    out=l1a[:], in0=l1a[:], scalar=NEG, in1=l1a[:],
                    op0=OP.mult, op1=OP.max)
                ps1b = mlpps.tile([128, 512], f32, tag="mlpp", space="PSUM")
                nc.tensor.matmul(out=ps1b[:], lhsT=w1bT[:], rhs=H[:, cs],
                                 start=True, stop=True)
                nc.scalar.activation(out=l1b[:], in_=ps1b[:],
                                     func=AF.Identity, bias=tm1b[:], scale=1.0)
                nc.vector.scalar_tensor_tensor(
                    out=l1b[:], in0=l1b[:], scalar=NEG, in1=l1b[:],
                    op0=OP.mult, op1=OP.max)
                ps2 = mlpps.tile([128, 512], f32, tag="mlpp", space="PSUM")
                nc.tensor.matmul(out=ps2[:], lhsT=w2maT[:], rhs=l1a[:],
                                 start=True, stop=False)
                nc.tensor.matmul(out=ps2[:], lhsT=w2mbT[:], rhs=l1b[:],
                                 start=False, stop=True)
                l2 = mlpsb.tile([128, 512], f32, tag="l2")
                nc.scalar.activation(out=l2[:], in_=ps2[:],
                                     func=AF.Identity, bias=tm2[:], scale=1.0)
                nc.vector.scalar_tensor_tensor(
                    out=l2[:], in0=l2[:], scalar=NEG, in1=l2[:],
                    op0=OP.mult, op1=OP.max)
                ps3 = mlpps.tile([1, 512], f32, tag="mlpp", space="PSUM")
                nc.tensor.matmul(out=ps3[:], lhsT=w3T[:], rhs=l2[:],
                                 start=True, stop=True)
                nc.scalar.activation(out=osb[:, cs], in_=ps3[:],
                                     func=AF.Identity, bias=b3[:], scale=1.0)
            nc.sync.dma_start(out_d[:], osb[:])

    nc.finalize()
    return nc


def host_weights(w_k1, g_k1, b_k1, m_k1, v_k1, w_k2, g_k2, b_k2, m_k2, v_k2,
                 w1, g1, b1, m1, v1, w2, g2, b2, m2, v2, w3, b3):
    f = np.float32
    s1 = (g_k1 / np.sqrt(v_k1 + f(EPS))).astype(f)
    t1 = (b_k1 - m_k1 * s1).astype(f)
    wn = w_k1[:, :C]
    wc = w_k1[:, C:]
    wnT = np.ascontiguousarray((wn * s1[:, None]).T.astype(f))
    wcnT = np.ascontiguousarray(((wc - wn) * s1[:, None]).T.astype(f))
    s2 = (g_k2 / np.sqrt(v_k2 + f(EPS))).astype(f)
    t2 = (b_k2 - m_k2 * s2).astype(f)
    w2T = np.ascontiguousarray((w_k2 * s2[:, None]).T.astype(f))
    sm1 = (g1 / np.sqrt(v1 + f(EPS))).astype(f)
    tm1 = (b1 - m1 * sm1).astype(f)
    w1s = (w1 * sm1[:, None]).astype(f)          # (256, 64)
    w1aT = np.ascontiguousarray(w1s[:128].T)      # (64, 128)
    w1bT = np.ascontiguousarray(w1s[128:].T)
    sm2 = (g2 / np.sqrt(v2 + f(EPS))).astype(f)
    tm2 = (b2 - m2 * sm2).astype(f)
    w2s = (w2 * sm2[:, None]).astype(f)          # (128, 256)
    w2maT = np.ascontiguousarray(w2s[:, :128].T)  # (128, 128)
    w2mbT = np.ascontiguousarray(w2s[:, 128:].T)
    w3T = np.ascontiguousarray(w3.T.astype(f))    # (128, 1)
    return {
        "wnT": wnT, "wcnT": wcnT, "t1": t1.reshape(C, 1),
        "w2T": w2T, "t2": t2.reshape(C, 1),
        "w1aT": w1aT, "w1bT": w1bT,
        "tm1a": tm1[:128].reshape(128, 1), "tm1b": tm1[128:].reshape(128, 1),
        "w2maT": w2maT, "w2mbT": w2mbT, "tm2": tm2.reshape(128, 1),
        "w3T": w3T, "b3": b3.reshape(1, 1).astype(f),
    }


def kernel(**inputs):
    from concourse.bass_utils import run_bass_kernel_spmd

    x = np.asarray(inputs["x"], dtype=np.float32)  # (B, C, N)
    B = x.shape[0]
    n = x.shape[2]
    w = host_weights(**{k: np.asarray(v, dtype=np.float32)
                        for k, v in inputs.items() if k != "x"})
    if n not in _cache:
        _cache[n] = build_nc(n)
    nc = _cache[n]
    in_maps = [{"x": np.ascontiguousarray(x[c]), **w} for c in range(B)]
    res = run_bass_kernel_spmd(nc, in_maps, list(range(NCORES)))
    out = np.stack([res.results[c]["out"][0] for c in range(B)], axis=0)
    return out.astype(np.float32)



# revision 2
# speedup vs baseline: 2.6163x; 2.6163x over previous
"""DGCNN prediction head on 8 Trainium2 NeuronCores.

Data-parallel over batch B=8: each core runs the full pipeline for one
sample (C=64 channels, N=4096 points, k=20 neighbors).

Per-core pipeline (all on one NeuronCore, no collectives):
  1. pairwise ranking R[i,j] = 2<x_i,x_j> - ||x_j||^2 via PE matmul with an
     augmented contract row; R stays in PSUM (no drain).
  2. top-20 per row, chunked: per 512-col chunk DVE max8 + max_index read
     the PSUM tile directly (2 passes over the row total instead of 8).
     The 64 chunk candidates are packed (value mantissa | column index) so
     a 3-round max8/match_replace on the 64-wide union yields the top-20
     indices with no further full-row scans.
  3. EdgeConv1 is linear before the LReLU, so it is precomputed per point:
       conv1(i,j) = Wn x_j + (Wc - Wn) x_i  with BN1 folded in
     A' = s1*(Wn x)        -> fp16 DRAM table, 20 rows gathered per block
                              in ONE indirect DMA
     B' = s1*((Wc-Wn) x)+t1-> fp16 on-chip, broadcast-added per query block
  4. e1 = lrelu(A'_j + B'_i): add on DVE (fp16 2x), lrelu on GPSIMD;
     PE-transpose to channel-major (fp16); EdgeConv2 as fp16 64x64 matmuls
     (BN2 scale folded into W2, bias t2 added during the PSUM drain);
     max over k as fp16 DVE tree; lrelu (monotone, s2 >= 0).
  5. point MLP 64->256->128->1, fp16 weights, BN scales folded, biases
     added during PSUM drains, lrelu on GPSIMD.
"""

import numpy as np

C = 64
K = 20
NEG = 0.2
EPS = 1e-5
NCORES = 8
N_FULL = 4096
NEG_FILL = -3.0e38
NCH = 8          # 512-col chunks per 128-row block
CHW = 512        # chunk width

_cache = {}


def build_nc(n):
    from contextlib import ExitStack

    import concourse.bass as bass
    import concourse.bacc as bacc
    import concourse.mybir as mybir
    import concourse.tile as tile
    from concourse.masks import make_identity

    f32 = mybir.dt.float32
    f16 = mybir.dt.float16
    u32 = mybir.dt.uint32
    AF = mybir.ActivationFunctionType
    OP = mybir.AluOpType

    nblk = n // 128
    nchk = n // 512

    nc = bacc.Bacc("TRN2", target_bir_lowering=False, debug=False,
                   num_devices=NCORES)

    x_d = nc.dram_tensor("x", [C, n], f32, kind="ExternalInput")
    wnT_d = nc.dram_tensor("wnT", [C, C], f32, kind="ExternalInput")
    wcnT_d = nc.dram_tensor("wcnT", [C, C], f32, kind="ExternalInput")
    t1_d = nc.dram_tensor("t1", [C, 1], f32, kind="ExternalInput")
    w2T_d = nc.dram_tensor("w2T", [C, C], f16, kind="ExternalInput")
    t2_d = nc.dram_tensor("t2", [C, 1], f32, kind="ExternalInput")
    w1aT_d = nc.dram_tensor("w1aT", [C, 128], f16, kind="ExternalInput")
    w1bT_d = nc.dram_tensor("w1bT", [C, 128], f16, kind="ExternalInput")
    tm1a_d = nc.dram_tensor("tm1a", [128, 1], f32, kind="ExternalInput")
    tm1b_d = nc.dram_tensor("tm1b", [128, 1], f32, kind="ExternalInput")
    w2maT_d = nc.dram_tensor("w2maT", [128, 128], f16, kind="ExternalInput")
    w2mbT_d = nc.dram_tensor("w2mbT", [128, 128], f16, kind="ExternalInput")
    tm2_d = nc.dram_tensor("tm2", [128, 1], f32, kind="ExternalInput")
    w3T_d = nc.dram_tensor("w3T", [128, 1], f16, kind="ExternalInput")
    b3_d = nc.dram_tensor("b3", [1, 1], f32, kind="ExternalInput")
    choff_d = nc.dram_tensor("choff", [128, NCH * 8], u32, kind="ExternalInput")
    out_d = nc.dram_tensor("out", [1, n], f32, kind="ExternalOutput")

    with tile.TileContext(nc) as tc, ExitStack() as top:
        cpool = top.enter_context(tc.tile_pool(name="consts", bufs=1))
        dpool = top.enter_context(tc.tile_pool(name="dram", bufs=1, space="DRAM"))
        xpool = top.enter_context(tc.tile_pool(name="xaug", bufs=1))
        hpool = top.enter_context(tc.tile_pool(name="hout", bufs=1))

        # --- constants / weights ---
        ident = cpool.tile([128, 128], f32, tag="ident")
        make_identity(nc, ident[:])
        ident16 = cpool.tile([128, 128], f16, tag="ident16")
        nc.scalar.copy(out=ident16[:], in_=ident[:])
        ones64 = cpool.tile([C, 1], f32, tag="ones64")
        nc.vector.memset(ones64[:], 1.0)

        def load_const(dram, shape, tag, dt=f32):
            t = cpool.tile(shape, dt, tag=tag)
            nc.sync.dma_start(t[:], dram[:])
            return t

        wnT = load_const(wnT_d, [C, C], "wnT")
        wcnT = load_const(wcnT_d, [C, C], "wcnT")
        t1 = load_const(t1_d, [C, 1], "t1")
        w2T = load_const(w2T_d, [C, C], "w2T", f16)
        t2 = load_const(t2_d, [C, 1], "t2")
        w1aT = load_const(w1aT_d, [C, 128], "w1aT", f16)
        w1bT = load_const(w1bT_d, [C, 128], "w1bT", f16)
        tm1a = load_const(tm1a_d, [128, 1], "tm1a")
        tm1b = load_const(tm1b_d, [128, 1], "tm1b")
        w2maT = load_const(w2maT_d, [128, 128], "w2maT", f16)
        w2mbT = load_const(w2mbT_d, [128, 128], "w2mbT", f16)
        tm2 = load_const(tm2_d, [128, 1], "tm2")
        w3T = load_const(w3T_d, [128, 1], "w3T", f16)
        b3 = load_const(b3_d, [1, 1], "b3")
        choff = load_const(choff_d, [128, NCH * 8], "choff", u32)

        At = dpool.tile([n, C], f16, tag="At")           # A' fp16 gather table
        xaug = xpool.tile([C + 1, n], f32, tag="xaug")   # rows 0..63 = x, row 64 = ||x_j||^2
        x2aug = xpool.tile([C + 1, n], f32, tag="x2aug") # rows 0..63 = 2x, row 64 = -1
        Bt = xpool.tile([128, C * nblk], f16, tag="Bt")  # B' fp16, block j at cols 64j
        H = hpool.tile([C, n], f16, tag="H")             # per-point features after edge max
        osb = hpool.tile([1, n], f32, tag="osb")

        # ---------------- stage 0: tables ----------------
        with tc.tile_pool(name="s0sb", bufs=2) as s0sb, \
             tc.tile_pool(name="s0ps", bufs=3, space="PSUM") as s0ps, \
             tc.tile_pool(name="s0pt", bufs=2, space="PSUM") as s0pt:
            nc.sync.dma_start(xaug[:C, :], x_d[:])
            nc.scalar.activation(out=x2aug[:C, :], in_=xaug[:C, :],
                                 func=AF.Copy, scale=2.0)
            nc.vector.memset(x2aug[C:C + 1, :], -1.0)
            for ch in range(nchk):
                cs = slice(512 * ch, 512 * (ch + 1))
                xsq = s0sb.tile([C, 512], f32, tag="xsq")
                nc.scalar.activation(out=xsq[:], in_=xaug[:C, cs], func=AF.Square)
                psxx = s0ps.tile([1, 512], f32, tag="s0p", space="PSUM")
                nc.tensor.matmul(out=psxx[:], lhsT=ones64[:], rhs=xsq[:],
                                 start=True, stop=True)
                nc.scalar.copy(out=xaug[C:C + 1, cs], in_=psxx[:])
            for ch in range(nchk):
                cs = slice(512 * ch, 512 * (ch + 1))
                psa = s0ps.tile([C, 512], f32, tag="s0p", space="PSUM")
                nc.tensor.matmul(out=psa[:], lhsT=wnT[:], rhs=xaug[:C, cs],
                                 start=True, stop=True)
                ap = s0sb.tile([C, 512], f16, tag="ap")
                nc.scalar.copy(out=ap[:], in_=psa[:])
                psb = s0ps.tile([C, 512], f32, tag="s0p", space="PSUM")
                nc.tensor.matmul(out=psb[:], lhsT=wcnT[:], rhs=xaug[:C, cs],
                                 start=True, stop=True)
                bp = s0sb.tile([C, 512], f16, tag="bp")
                nc.scalar.activation(out=bp[:], in_=psb[:], func=AF.Identity,
                                     bias=t1[:], scale=1.0)
                ast = s0sb.tile([128, 4 * C], f16, tag="ast")
                for j in range(4):
                    blk = 4 * ch + j
                    js = slice(128 * j, 128 * (j + 1))
                    pta = s0pt.tile([128, C], f16, tag="s0t", space="PSUM")
                    nc.tensor.transpose(out=pta[:], in_=ap[:, js],
                                        identity=ident16[:C, :C])
                    nc.scalar.copy(out=ast[:, C * j:C * (j + 1)], in_=pta[:])
                    ptb = s0pt.tile([128, C], f16, tag="s0t", space="PSUM")
                    nc.tensor.transpose(out=ptb[:], in_=bp[:, js],
                                        identity=ident16[:C, :C])
                    nc.scalar.copy(out=Bt[:, C * blk:C * (blk + 1)], in_=ptb[:])
                nc.sync.dma_start(
                    At[512 * ch:512 * (ch + 1), :].rearrange(
                        "(j p) c -> p j c", p=128),
                    ast[:].rearrange("p (j c) -> p j c", j=4))

        # ---------------- stage 1: blocks ----------------
        with tc.tile_pool(name="cpoolv", bufs=2) as cvp, \
             tc.tile_pool(name="gpool", bufs=2) as gpool, \
             tc.tile_pool(name="epool", bufs=2) as epool, \
             tc.tile_pool(name="wpool", bufs=2) as wpool, \
             tc.tile_pool(name="tpool", bufs=2) as tpool, \
             tc.tile_pool(name="psR", bufs=3, space="PSUM") as psR, \
             tc.tile_pool(name="psT", bufs=2, space="PSUM") as psT, \
             tc.tile_pool(name="psE", bufs=2, space="PSUM") as psE:

            for b in range(nblk):
                bs = slice(128 * b, 128 * (b + 1))

                # pairwise + chunked top-8, straight from PSUM
                cand_v = cvp.tile([128, NCH * 8], f32, tag="cv")
                cand_i = cvp.tile([128, NCH * 8], u32, tag="ci")
                for ch in range(NCH):
                    cs = slice(CHW * ch, CHW * (ch + 1))
                    ks = slice(8 * ch, 8 * (ch + 1))
                    ps = psR.tile([128, CHW], f32, tag="psr", space="PSUM")
                    nc.tensor.matmul(out=ps[:], lhsT=x2aug[:, bs],
                                     rhs=xaug[:, cs], start=True, stop=True)
                    nc.vector.max(out=cand_v[:, ks], in_=ps[:])
                    nc.vector.max_index(out=cand_i[:, ks],
                                        in_max=cand_v[:, ks], in_values=ps[:])

                # pack value|index on GPSIMD, union top-20 on DVE
                pk = cvp.tile([128, NCH * 8], u32, tag="pk")
                nc.gpsimd.tensor_tensor(out=cand_i[:], in0=cand_i[:],
                                        in1=choff[:], op=OP.add)
                nc.gpsimd.tensor_scalar(out=pk[:], in0=cand_v[:].bitcast(u32),
                                        scalar1=0xFFFFF000, scalar2=None,
                                        op0=OP.bitwise_and)
                nc.gpsimd.tensor_tensor(out=pk[:], in0=pk[:], in1=cand_i[:],
                                        op=OP.bitwise_or)
                pkf = pk[:].bitcast(f32)
                pv1 = cvp.tile([128, 8], f32, tag="pv1")
                pv2 = cvp.tile([128, 8], f32, tag="pv2")
                pv3 = cvp.tile([128, 8], f32, tag="pv3")
                nc.vector.max(out=pv1[:], in_=pkf)
                nc.vector.match_replace(out=pkf, in_to_replace=pv1[:],
                                        in_values=pkf, imm_value=NEG_FILL)
                nc.vector.max(out=pv2[:], in_=pkf)
                nc.vector.match_replace(out=pkf, in_to_replace=pv2[:],
                                        in_values=pkf, imm_value=NEG_FILL)
                nc.vector.max(out=pv3[:], in_=pkf)
                idx = cvp.tile([128, 24], u32, tag="idx")
                nc.gpsimd.tensor_scalar(out=idx[:, 0:8], in0=pv1[:].bitcast(u32),
                                        scalar1=0xFFF, scalar2=None,
                                        op0=OP.bitwise_and)
                nc.gpsimd.tensor_scalar(out=idx[:, 8:16], in0=pv2[:].bitcast(u32),
                                        scalar1=0xFFF, scalar2=None,
                                        op0=OP.bitwise_and)
                nc.gpsimd.tensor_scalar(out=idx[:, 16:24], in0=pv3[:].bitcast(u32),
                                        scalar1=0xFFF, scalar2=None,
                                        op0=OP.bitwise_and)

                # gather all 20 neighbors in one indirect DMA (fp16 rows)
                G = gpool.tile([128, K * C], f16, tag="G")
                nc.gpsimd.indirect_dma_start(
                    out=G[:].rearrange("p (k c) -> p k c", k=K),
                    out_offset=None,
                    in_=At[:],
                    in_offset=bass.IndirectOffsetOnAxis(ap=idx[:, 0:K], axis=0))

                # e1 = lrelu(G + B'_i): add on DVE (fp16 2x), lrelu on GPSIMD
                bb = Bt[:, C * b:C * (b + 1)].rearrange(
                    "p (k c) -> p k c", k=1).to_broadcast([128, K, C])
                nc.vector.tensor_tensor(
                    out=G[:].rearrange("p (k c) -> p k c", k=K),
                    in0=G[:].rearrange("p (k c) -> p k c", k=K),
                    in1=bb, op=OP.add)
                nc.gpsimd.scalar_tensor_tensor(
                    out=G[:], in0=G[:], scalar=NEG, in1=G[:],
                    op0=OP.mult, op1=OP.max)

                # transpose to channel-major: 20 PE transposes [128,64]->[64,128]
                e1T = epool.tile([C, K * 128], f16, tag="e1T")
                for grp in range(5):
                    pt = psT.tile([C, 512], f16, tag="pst", space="PSUM")
                    for s in range(4):
                        k = 4 * grp + s
                        nc.tensor.transpose(
                            out=pt[:, 128 * s:128 * (s + 1)],
                            in_=G[:, C * k:C * (k + 1)],
                            identity=ident16[:])
                    nc.scalar.copy(out=e1T[:, 512 * grp:512 * (grp + 1)],
                                   in_=pt[:])

                # conv2 (w_k2 with bn2 scale folded), t2 added in drain
                ew = wpool.tile([C, K * 128], f16, tag="ew")
                for grp in range(5):
                    pe = psE.tile([C, 512], f32, tag="pse", space="PSUM")
                    for s in range(4):
                        k = 4 * grp + s
                        nc.tensor.matmul(
                            out=pe[:, 128 * s:128 * (s + 1)],
                            lhsT=w2T[:],
                            rhs=e1T[:, 128 * k:128 * (k + 1)],
                            start=True, stop=True)
                    nc.scalar.activation(
                        out=ew[:, 512 * grp:512 * (grp + 1)], in_=pe[:],
                        func=AF.Identity, bias=t2[:], scale=1.0)

                # max over k (fp16 DVE tree), then lrelu -> H
                m1 = tpool.tile([C, 10 * 128], f16, tag="m1")
                nc.vector.tensor_tensor(out=m1[:], in0=ew[:, :1280],
                                        in1=ew[:, 1280:], op=OP.max)
                m2 = tpool.tile([C, 5 * 128], f16, tag="m2")
                nc.vector.tensor_tensor(out=m2[:], in0=m1[:, :640],
                                        in1=m1[:, 640:], op=OP.max)
                m3 = tpool.tile([C, 2 * 128], f16, tag="m3")
                nc.vector.tensor_tensor(out=m3[:], in0=m2[:, :256],
                                        in1=m2[:, 256:512], op=OP.max)
                m4 = tpool.tile([C, 128], f16, tag="m4")
                nc.vector.tensor_tensor(out=m4[:], in0=m3[:, :128],
                                        in1=m3[:, 128:], op=OP.max)
                nc.vector.tensor_tensor(out=m4[:], in0=m4[:],
                                        in1=m2[:, 512:], op=OP.max)
                nc.vector.scalar_tensor_tensor(
                    out=H[:, bs], in0=m4[:], scalar=NEG, in1=m4[:],
                    op0=OP.mult, op1=OP.max)

        # ---------------- stage 2: point MLP ----------------
        with tc.tile_pool(name="mlpsb", bufs=2) as mlpsb, \
             tc.tile_pool(name="mlpps", bufs=4, space="PSUM") as mlpps:
            for ch in range(nchk):
                cs = slice(512 * ch, 512 * (ch + 1))
                l1a = mlpsb.tile([128, 512], f16, tag="l1a")
                l1b = mlpsb.tile([128, 512], f16, tag="l1b")
                ps1a = mlpps.tile([128, 512], f32, tag="mlpp", space="PSUM")
                nc.tensor.matmul(out=ps1a[:], lhsT=w1aT[:], rhs=H[:, cs],
                                 start=True, stop=True)
                nc.scalar.activation(out=l1a[:], in_=ps1a[:],
                                     func=AF.Identity, bias=tm1a[:], scale=1.0)
                nc.gpsimd.scalar_tensor_tensor(
                    out=l1a[:], in0=l1a[:], scalar=NEG, in1=l1a[:],
                    op0=OP.mult, op1=OP.max)
                ps1b = mlpps.tile([128, 512], f32, tag="mlpp", space="PSUM")
                nc.tensor.matmul(out=ps1b[:], lhsT=w1bT[:], rhs=H[:, cs],
                                 start=True, stop=True)
                nc.scalar.activation(out=l1b[:], in_=ps1b[:],
                                     func=AF.Identity, bias=tm1b[:], scale=1.0)
                nc.gpsimd.scalar_tensor_tensor(
                    out=l1b[:], in0=l1b[:], scalar=NEG, in1=l1b[:],
                    op0=OP.mult, op1=OP.max)
                ps2 = mlpps.tile([128, 512], f32, tag="mlpp", space="PSUM")
                nc.tensor.matmul(out=ps2[:], lhsT=w2maT[:], rhs=l1a[:],
                                 start=True, stop=False)
                nc.tensor.matmul(out=ps2[:], lhsT=w2mbT[:], rhs=l1b[:],
                                 start=False, stop=True)
                l2 = mlpsb.tile([128, 512], f16, tag="l2")
                nc.scalar.activation(out=l2[:], in_=ps2[:],
                                     func=AF.Identity, bias=tm2[:], scale=1.0)
                nc.gpsimd.scalar_tensor_tensor(
                    out=l2[:], in0=l2[:], scalar=NEG, in1=l2[:],
                    op0=OP.mult, op1=OP.max)
                ps3 = mlpps.tile([1, 512], f32, tag="mlpp", space="PSUM")
                nc.tensor.matmul(out=ps3[:], lhsT=w3T[:], rhs=l2[:],
                                 start=True, stop=True)
                nc.scalar.activation(out=osb[:, cs], in_=ps3[:],
                                     func=AF.Identity, bias=b3[:], scale=1.0)
            nc.sync.dma_start(out_d[:], osb[:])

    nc.finalize()
    return nc


def host_weights(w_k1, g_k1, b_k1, m_k1, v_k1, w_k2, g_k2, b_k2, m_k2, v_k2,
                 w1, g1, b1, m1, v1, w2, g2, b2, m2, v2, w3, b3):
    f = np.float32
    h = np.float16
    s1 = (g_k1 / np.sqrt(v_k1 + f(EPS))).astype(f)
    t1 = (b_k1 - m_k1 * s1).astype(f)
    wn = w_k1[:, :C]
    wc = w_k1[:, C:]
    wnT = np.ascontiguousarray((wn * s1[:, None]).T.astype(f))
    wcnT = np.ascontiguousarray(((wc - wn) * s1[:, None]).T.astype(f))
    s2 = (g_k2 / np.sqrt(v_k2 + f(EPS))).astype(f)
    t2 = (b_k2 - m_k2 * s2).astype(f)
    w2T = np.ascontiguousarray((w_k2 * s2[:, None]).T.astype(h))
    sm1 = (g1 / np.sqrt(v1 + f(EPS))).astype(f)
    tm1 = (b1 - m1 * sm1).astype(f)
    w1s = (w1 * sm1[:, None]).astype(f)            # (256, 64)
    w1aT = np.ascontiguousarray(w1s[:128].T.astype(h))  # (64, 128)
    w1bT = np.ascontiguousarray(w1s[128:].T.astype(h))
    sm2 = (g2 / np.sqrt(v2 + f(EPS))).astype(f)
    tm2 = (b2 - m2 * sm2).astype(f)
    w2s = (w2 * sm2[:, None]).astype(f)            # (128, 256)
    w2maT = np.ascontiguousarray(w2s[:, :128].T.astype(h))  # (128, 128)
    w2mbT = np.ascontiguousarray(w2s[:, 128:].T.astype(h))
    w3T = np.ascontiguousarray(w3.T.astype(h))     # (128, 1)
    choff = np.broadcast_to(
        (np.repeat(np.arange(NCH, dtype=np.uint32), 8) * CHW)[None, :],
        (128, NCH * 8))
    return {
        "wnT": wnT, "wcnT": wcnT, "t1": t1.reshape(C, 1),
        "w2T": w2T, "t2": t2.reshape(C, 1),
        "w1aT": w1aT, "w1bT": w1bT,
        "tm1a": tm1[:128].reshape(128, 1), "tm1b": tm1[128:].reshape(128, 1),
        "w2maT": w2maT, "w2mbT": w2mbT, "tm2": tm2.reshape(128, 1),
        "w3T": w3T, "b3": b3.reshape(1, 1).astype(f),
        "choff": np.ascontiguousarray(choff),
    }


def kernel(**inputs):
    from concourse.bass_utils import run_bass_kernel_spmd

    x = np.asarray(inputs["x"], dtype=np.float32)  # (B, C, N)
    B = x.shape[0]
    n = x.shape[2]
    w = host_weights(**{k: np.asarray(v, dtype=np.float32)
                        for k, v in inputs.items() if k != "x"})
    if n not in _cache:
        _cache[n] = build_nc(n)
    nc = _cache[n]
    in_maps = [{"x": np.ascontiguousarray(x[c]), **w} for c in range(B)]
    res = run_bass_kernel_spmd(nc, in_maps, list(range(NCORES)))
    out = np.stack([res.results[c]["out"][0] for c in range(B)], axis=0)
    return out.astype(np.float32)


# revision 9
# speedup vs baseline: 2.9520x; 1.1283x over previous
"""DGCNN prediction head on 8 Trainium2 NeuronCores.

Data-parallel over batch B=8: each core runs the full pipeline for one
sample (C=64 channels, N=4096 points, k=20 neighbors).

Per-core pipeline (all on one NeuronCore, no collectives):
  1. pairwise ranking R[i,j] = 2<x_i,x_j> - ||x_j||^2 via PE matmul with an
     augmented contract row; R stays in PSUM (no drain).
  2. top-20 per row, chunked: per 512-col chunk DVE max8 + max_index read
     the PSUM tile directly (2 passes over the row total instead of 8).
     The 64 chunk candidates are packed (value mantissa | column index) so
     a 3-round max8/match_replace on the 64-wide union yields the top-20
     indices with no further full-row scans.
  3. EdgeConv1 is linear before the LReLU, so it is precomputed per point:
       conv1(i,j) = Wn x_j + (Wc - Wn) x_i  with BN1 folded in
     A' = s1*(Wn x)        -> fp16 DRAM table, 20 rows gathered per block
                              in ONE indirect DMA
     B' = s1*((Wc-Wn) x)+t1-> fp16 on-chip, broadcast-added per query block
  4. e1 = lrelu(A'_j + B'_i): add on DVE (fp16 2x), lrelu on GPSIMD;
     PE-transpose to channel-major (fp16); EdgeConv2 as fp16 64x64 matmuls
     (BN2 scale folded into W2, bias t2 added during the PSUM drain);
     max over k as fp16 DVE tree; lrelu (monotone, s2 >= 0).
  5. point MLP 64->256->128->1, fp16 weights, BN scales folded, biases
     added during PSUM drains, lrelu on GPSIMD.
"""

import numpy as np

C = 64
K = 20
NEG = 0.2
EPS = 1e-5
NCORES = 8
N_FULL = 4096
NEG_FILL = -3.0e38
NCH = 8          # 512-col chunks per 128-row block
CHW = 512        # chunk width

_cache = {}


def build_nc(n):
    from contextlib import ExitStack

    import concourse.bass as bass
    import concourse.bacc as bacc
    import concourse.mybir as mybir
    import concourse.tile as tile
    from concourse.masks import make_identity

    f32 = mybir.dt.float32
    f32r = mybir.dt.float32r
    f16 = mybir.dt.float16
    u32 = mybir.dt.uint32
    AF = mybir.ActivationFunctionType
    OP = mybir.AluOpType

    nblk = n // 128
    nchk = n // 512

    nc = bacc.Bacc("TRN2", target_bir_lowering=False, debug=False,
                   num_devices=NCORES)

    x_d = nc.dram_tensor("x", [C, n], f32, kind="ExternalInput")
    wnT_d = nc.dram_tensor("wnT", [C, C], f32, kind="ExternalInput")
    wcnT_d = nc.dram_tensor("wcnT", [C, C], f32, kind="ExternalInput")
    t1_d = nc.dram_tensor("t1", [C, 1], f32, kind="ExternalInput")
    w2T_d = nc.dram_tensor("w2T", [C, C], f16, kind="ExternalInput")
    t2_d = nc.dram_tensor("t2", [C, 1], f32, kind="ExternalInput")
    w1aT_d = nc.dram_tensor("w1aT", [C, 128], f16, kind="ExternalInput")
    w1bT_d = nc.dram_tensor("w1bT", [C, 128], f16, kind="ExternalInput")
    tm1a_d = nc.dram_tensor("tm1a", [128, 1], f32, kind="ExternalInput")
    tm1b_d = nc.dram_tensor("tm1b", [128, 1], f32, kind="ExternalInput")
    w2maT_d = nc.dram_tensor("w2maT", [128, 128], f16, kind="ExternalInput")
    w2mbT_d = nc.dram_tensor("w2mbT", [128, 128], f16, kind="ExternalInput")
    tm2_d = nc.dram_tensor("tm2", [128, 1], f32, kind="ExternalInput")
    w3T_d = nc.dram_tensor("w3T", [128, 1], f16, kind="ExternalInput")
    b3_d = nc.dram_tensor("b3", [1, 1], f32, kind="ExternalInput")
    choff_d = nc.dram_tensor("choff", [128, NCH * 8], u32, kind="ExternalInput")
    negrow_d = nc.dram_tensor("negrow", [1, n], f32, kind="ExternalInput")
    out_d = nc.dram_tensor("out", [1, n], f32, kind="ExternalOutput")

    with tile.TileContext(nc) as tc, ExitStack() as top:
        cpool = top.enter_context(tc.tile_pool(name="consts", bufs=1))
        dpool = top.enter_context(tc.tile_pool(name="dram", bufs=1, space="DRAM"))
        xpool = top.enter_context(tc.tile_pool(name="xaug", bufs=1))
        hpool = top.enter_context(tc.tile_pool(name="hout", bufs=1))

        # --- constants / weights ---
        ident = cpool.tile([128, 128], f32, tag="ident")
        make_identity(nc, ident[:])
        ident16 = cpool.tile([128, 128], f16, tag="ident16")
        nc.scalar.copy(out=ident16[:], in_=ident[:])
        ones64 = cpool.tile([C, 1], f32, tag="ones64")
        nc.vector.memset(ones64[:], 1.0)

        def load_const(dram, shape, tag, dt=f32):
            t = cpool.tile(shape, dt, tag=tag)
            nc.sync.dma_start(t[:], dram[:])
            return t

        wnT = load_const(wnT_d, [C, C], "wnT")
        wcnT = load_const(wcnT_d, [C, C], "wcnT")
        t1 = load_const(t1_d, [C, 1], "t1")
        w2T = load_const(w2T_d, [C, C], "w2T", f16)
        t2 = load_const(t2_d, [C, 1], "t2")
        w1aT = load_const(w1aT_d, [C, 128], "w1aT", f16)
        w1bT = load_const(w1bT_d, [C, 128], "w1bT", f16)
        tm1a = load_const(tm1a_d, [128, 1], "tm1a")
        tm1b = load_const(tm1b_d, [128, 1], "tm1b")
        w2maT = load_const(w2maT_d, [128, 128], "w2maT", f16)
        w2mbT = load_const(w2mbT_d, [128, 128], "w2mbT", f16)
        tm2 = load_const(tm2_d, [128, 1], "tm2")
        w3T = load_const(w3T_d, [128, 1], "w3T", f16)
        b3 = load_const(b3_d, [1, 1], "b3")
        choff = load_const(choff_d, [128, NCH * 8], "choff", u32)

        At = dpool.tile([n, C], f16, tag="At")           # A' fp16 gather table
        xaug = xpool.tile([C + 1, n], f32, tag="xaug")   # rows 0..63 = x, row 64 = ||x_j||^2
        x2aug = xpool.tile([C + 1, n], f32, tag="x2aug") # rows 0..63 = 2x, row 64 = -1
        Bt = xpool.tile([128, C * nblk], f16, tag="Bt")  # B' fp16, block j at cols 64j
        H = hpool.tile([C, n], f16, tag="H")             # per-point features after edge max
        osb = hpool.tile([1, n], f32, tag="osb")

        # ---------------- stage 0: tables ----------------
        with tc.tile_pool(name="s0sb", bufs=2) as s0sb, \
             tc.tile_pool(name="s0ps", bufs=3, space="PSUM") as s0ps, \
             tc.tile_pool(name="s0pt", bufs=2, space="PSUM") as s0pt:
            nc.sync.dma_start(xaug[:C, :], x_d[:])
            nc.sync.dma_start(x2aug[C:C + 1, :], negrow_d[:])
            nc.scalar.activation(out=x2aug[:C, :], in_=xaug[:C, :],
                                 func=AF.Copy, scale=2.0)
            for ch in range(nchk):
                cs = slice(512 * ch, 512 * (ch + 1))
                xsq = s0sb.tile([C, 512], f32, tag="xsq")
                nc.scalar.activation(out=xsq[:], in_=xaug[:C, cs], func=AF.Square)
                psxx = s0ps.tile([1, 512], f32, tag="s0p", space="PSUM")
                nc.tensor.matmul(out=psxx[:], lhsT=ones64[:].bitcast(f32r),
                                 rhs=xsq[:].bitcast(f32r),
                                 start=True, stop=True)
                nc.scalar.copy(out=xaug[C:C + 1, cs], in_=psxx[:])
            for ch in range(nchk):
                cs = slice(512 * ch, 512 * (ch + 1))
                psa = s0ps.tile([C, 512], f32, tag="s0p", space="PSUM")
                nc.tensor.matmul(out=psa[:], lhsT=wnT[:].bitcast(f32r),
                                 rhs=xaug[:C, cs].bitcast(f32r),
                                 start=True, stop=True)
                ap = s0sb.tile([C, 512], f16, tag="ap")
                nc.scalar.copy(out=ap[:], in_=psa[:])
                psb = s0ps.tile([C, 512], f32, tag="s0p", space="PSUM")
                nc.tensor.matmul(out=psb[:], lhsT=wcnT[:].bitcast(f32r),
                                 rhs=xaug[:C, cs].bitcast(f32r),
                                 start=True, stop=True)
                bp = s0sb.tile([C, 512], f16, tag="bp")
                nc.scalar.activation(out=bp[:], in_=psb[:], func=AF.Identity,
                                     bias=t1[:], scale=1.0)
                ast = s0sb.tile([128, 4 * C], f16, tag="ast")
                for j in range(4):
                    blk = 4 * ch + j
                    js = slice(128 * j, 128 * (j + 1))
                    pta = s0pt.tile([128, C], f16, tag="s0t", space="PSUM")
                    nc.tensor.transpose(out=pta[:], in_=ap[:, js],
                                        identity=ident16[:C, :C])
                    nc.scalar.copy(out=ast[:, C * j:C * (j + 1)], in_=pta[:])
                    ptb = s0pt.tile([128, C], f16, tag="s0t", space="PSUM")
                    nc.tensor.transpose(out=ptb[:], in_=bp[:, js],
                                        identity=ident16[:C, :C])
                    nc.scalar.copy(out=Bt[:, C * blk:C * (blk + 1)], in_=ptb[:])
                nc.sync.dma_start(
                    At[512 * ch:512 * (ch + 1), :].rearrange(
                        "(j p) c -> p j c", p=128),
                    ast[:].rearrange("p (j c) -> p j c", j=4))

        # ---------------- stage 1: blocks ----------------
        with tc.tile_pool(name="cpoolv", bufs=2) as cvp, \
             tc.tile_pool(name="gpool", bufs=2) as gpool, \
             tc.tile_pool(name="epool", bufs=2) as epool, \
             tc.tile_pool(name="wpool", bufs=2) as wpool, \
             tc.tile_pool(name="tpool", bufs=2) as tpool, \
             tc.tile_pool(name="psR", bufs=3, space="PSUM") as psR, \
             tc.tile_pool(name="psT", bufs=2, space="PSUM") as psT, \
             tc.tile_pool(name="psE", bufs=2, space="PSUM") as psE:

            for b in range(nblk):
                bs = slice(128 * b, 128 * (b + 1))

                # pairwise + chunked top-8, straight from PSUM
                cand_v = cvp.tile([128, NCH * 8], f32, tag="cv")
                cand_i = cvp.tile([128, NCH * 8], u32, tag="ci")
                for ch in range(NCH):
                    cs = slice(CHW * ch, CHW * (ch + 1))
                    ks = slice(8 * ch, 8 * (ch + 1))
                    ps = psR.tile([128, CHW], f32, tag="psr", space="PSUM")
                    nc.tensor.matmul(out=ps[:], lhsT=x2aug[:, bs].bitcast(f32r),
                                     rhs=xaug[:, cs].bitcast(f32r),
                                     start=True, stop=True)
                    nc.vector.max(out=cand_v[:, ks], in_=ps[:])
                    nc.vector.max_index(out=cand_i[:, ks],
                                        in_max=cand_v[:, ks], in_values=ps[:])

                # pack value|index on GPSIMD, union top-20 on DVE
                pk = cvp.tile([128, NCH * 8], u32, tag="pk")
                nc.gpsimd.tensor_tensor(out=cand_i[:], in0=cand_i[:],
                                        in1=choff[:], op=OP.add)
                nc.gpsimd.tensor_scalar(out=pk[:], in0=cand_v[:].bitcast(u32),
                                        scalar1=0xFFFFF000, scalar2=None,
                                        op0=OP.bitwise_and)
                nc.gpsimd.tensor_tensor(out=pk[:], in0=pk[:], in1=cand_i[:],
                                        op=OP.bitwise_or)
                pkf = pk[:].bitcast(f32)
                pv1 = cvp.tile([128, 8], f32, tag="pv1")
                pv2 = cvp.tile([128, 8], f32, tag="pv2")
                pv3 = cvp.tile([128, 8], f32, tag="pv3")
                nc.vector.max(out=pv1[:], in_=pkf)
                nc.vector.match_replace(out=pkf, in_to_replace=pv1[:],
                                        in_values=pkf, imm_value=NEG_FILL)
                nc.vector.max(out=pv2[:], in_=pkf)
                nc.vector.match_replace(out=pkf, in_to_replace=pv2[:],
                                        in_values=pkf, imm_value=NEG_FILL)
                nc.vector.max(out=pv3[:], in_=pkf)
                idx = cvp.tile([128, 24], u32, tag="idx")
                nc.gpsimd.tensor_scalar(out=idx[:, 0:8], in0=pv1[:].bitcast(u32),
                                        scalar1=0xFFF, scalar2=None,
                                        op0=OP.bitwise_and)
                nc.gpsimd.tensor_scalar(out=idx[:, 8:16], in0=pv2[:].bitcast(u32),
                                        scalar1=0xFFF, scalar2=None,
                                        op0=OP.bitwise_and)
                nc.gpsimd.tensor_scalar(out=idx[:, 16:24], in0=pv3[:].bitcast(u32),
                                        scalar1=0xFFF, scalar2=None,
                                        op0=OP.bitwise_and)

                # gather all 20 neighbors in one indirect DMA (fp16 rows)
                G = gpool.tile([128, K * C], f16, tag="G")
                nc.gpsimd.indirect_dma_start(
                    out=G[:].rearrange("p (k c) -> p k c", k=K),
                    out_offset=None,
                    in_=At[:],
                    in_offset=bass.IndirectOffsetOnAxis(ap=idx[:, 0:K], axis=0))

                # e1 = lrelu(G + B'_i): add + lrelu both on GPSIMD
                bb = Bt[:, C * b:C * (b + 1)].rearrange(
                    "p (k c) -> p k c", k=1).to_broadcast([128, K, C])
                nc.gpsimd.tensor_tensor(
                    out=G[:].rearrange("p (k c) -> p k c", k=K),
                    in0=G[:].rearrange("p (k c) -> p k c", k=K),
                    in1=bb, op=OP.add)
                nc.gpsimd.scalar_tensor_tensor(
                    out=G[:], in0=G[:], scalar=NEG, in1=G[:],
                    op0=OP.mult, op1=OP.max)

                # transpose to channel-major: 20 PE transposes [128,64]->[64,128]
                e1T = epool.tile([C, K * 128], f16, tag="e1T")
                for grp, gw in ((0, 8), (1, 8), (2, 4)):
                    pt = psT.tile([C, 1024], f16, tag="pst", space="PSUM")
                    for s in range(gw):
                        k = 8 * grp + s
                        nc.tensor.transpose(
                            out=pt[:, 128 * s:128 * (s + 1)],
                            in_=G[:, C * k:C * (k + 1)],
                            identity=ident16[:])
                    nc.scalar.copy(
                        out=e1T[:, 1024 * grp:1024 * grp + 128 * gw],
                        in_=pt[:, :128 * gw])

                # conv2 (w_k2 with bn2 scale folded), t2 added in drain
                ew = wpool.tile([C, K * 128], f16, tag="ew")
                for grp in range(5):
                    pe = psE.tile([C, 512], f32, tag="pse", space="PSUM")
                    for s in range(4):
                        k = 4 * grp + s
                        nc.tensor.matmul(
                            out=pe[:, 128 * s:128 * (s + 1)],
                            lhsT=w2T[:],
                            rhs=e1T[:, 128 * k:128 * (k + 1)],
                            start=True, stop=True)
                    nc.scalar.activation(
                        out=ew[:, 512 * grp:512 * (grp + 1)], in_=pe[:],
                        func=AF.Identity, bias=t2[:], scale=1.0)

                # max over k: levels 1-2 on GPSIMD, rest + lrelu on DVE
                m1 = tpool.tile([C, 10 * 128], f16, tag="m1")
                nc.gpsimd.tensor_tensor(out=m1[:], in0=ew[:, :1280],
                                        in1=ew[:, 1280:], op=OP.max)
                m2 = tpool.tile([C, 5 * 128], f16, tag="m2")
                nc.gpsimd.tensor_tensor(out=m2[:], in0=m1[:, :640],
                                        in1=m1[:, 640:], op=OP.max)
                m3 = tpool.tile([C, 2 * 128], f16, tag="m3")
                nc.vector.tensor_tensor(out=m3[:], in0=m2[:, :256],
                                        in1=m2[:, 256:512], op=OP.max)
                m4 = tpool.tile([C, 128], f16, tag="m4")
                nc.vector.tensor_tensor(out=m4[:], in0=m3[:, :128],
                                        in1=m3[:, 128:], op=OP.max)
                nc.vector.tensor_tensor(out=m4[:], in0=m4[:],
                                        in1=m2[:, 512:], op=OP.max)
                nc.vector.scalar_tensor_tensor(
                    out=H[:, bs], in0=m4[:], scalar=NEG, in1=m4[:],
                    op0=OP.mult, op1=OP.max)

        # ---------------- stage 2: point MLP ----------------
        with tc.tile_pool(name="mlpsb", bufs=2) as mlpsb, \
             tc.tile_pool(name="mlpps", bufs=4, space="PSUM") as mlpps:
            for ch in range(nchk):
                cs = slice(512 * ch, 512 * (ch + 1))
                l1a = mlpsb.tile([128, 512], f16, tag="l1a")
                l1b = mlpsb.tile([128, 512], f16, tag="l1b")
                ps1a = mlpps.tile([128, 512], f32, tag="mlpp", space="PSUM")
                nc.tensor.matmul(out=ps1a[:], lhsT=w1aT[:], rhs=H[:, cs],
                                 start=True, stop=True)
                nc.scalar.activation(out=l1a[:], in_=ps1a[:],
                                     func=AF.Identity, bias=tm1a[:], scale=1.0)
                nc.gpsimd.scalar_tensor_tensor(
                    out=l1a[:], in0=l1a[:], scalar=NEG, in1=l1a[:],
                    op0=OP.mult, op1=OP.max)
                ps1b = mlpps.tile([128, 512], f32, tag="mlpp", space="PSUM")
                nc.tensor.matmul(out=ps1b[:], lhsT=w1bT[:], rhs=H[:, cs],
                                 start=True, stop=True)
                nc.scalar.activation(out=l1b[:], in_=ps1b[:],
                                     func=AF.Identity, bias=tm1b[:], scale=1.0)
                nc.gpsimd.scalar_tensor_tensor(
                    out=l1b[:], in0=l1b[:], scalar=NEG, in1=l1b[:],
                    op0=OP.mult, op1=OP.max)
                ps2 = mlpps.tile([128, 512], f32, tag="mlpp", space="PSUM")
                nc.tensor.matmul(out=ps2[:], lhsT=w2maT[:], rhs=l1a[:],
                                 start=True, stop=False)
                nc.tensor.matmul(out=ps2[:], lhsT=w2mbT[:], rhs=l1b[:],
                                 start=False, stop=True)
                l2 = mlpsb.tile([128, 512], f16, tag="l2")
                nc.scalar.activation(out=l2[:], in_=ps2[:],
                                     func=AF.Identity, bias=tm2[:], scale=1.0)
                nc.gpsimd.scalar_tensor_tensor(
                    out=l2[:], in0=l2[:], scalar=NEG, in1=l2[:],
                    op0=OP.mult, op1=OP.max)
                ps3 = mlpps.tile([1, 512], f32, tag="mlpp", space="PSUM")
                nc.tensor.matmul(out=ps3[:], lhsT=w3T[:], rhs=l2[:],
                                 start=True, stop=True)
                nc.scalar.activation(out=osb[:, cs], in_=ps3[:],
                                     func=AF.Identity, bias=b3[:], scale=1.0)
            nc.sync.dma_start(out_d[:], osb[:])

    nc.finalize()
    return nc


def host_weights(w_k1, g_k1, b_k1, m_k1, v_k1, w_k2, g_k2, b_k2, m_k2, v_k2,
                 w1, g1, b1, m1, v1, w2, g2, b2, m2, v2, w3, b3):
    f = np.float32
    h = np.float16
    s1 = (g_k1 / np.sqrt(v_k1 + f(EPS))).astype(f)
    t1 = (b_k1 - m_k1 * s1).astype(f)
    wn = w_k1[:, :C]
    wc = w_k1[:, C:]
    wnT = np.ascontiguousarray((wn * s1[:, None]).T.astype(f))
    wcnT = np.ascontiguousarray(((wc - wn) * s1[:, None]).T.astype(f))
    s2 = (g_k2 / np.sqrt(v_k2 + f(EPS))).astype(f)
    t2 = (b_k2 - m_k2 * s2).astype(f)
    w2T = np.ascontiguousarray((w_k2 * s2[:, None]).T.astype(h))
    sm1 = (g1 / np.sqrt(v1 + f(EPS))).astype(f)
    tm1 = (b1 - m1 * sm1).astype(f)
    w1s = (w1 * sm1[:, None]).astype(f)            # (256, 64)
    w1aT = np.ascontiguousarray(w1s[:128].T.astype(h))  # (64, 128)
    w1bT = np.ascontiguousarray(w1s[128:].T.astype(h))
    sm2 = (g2 / np.sqrt(v2 + f(EPS))).astype(f)
    tm2 = (b2 - m2 * sm2).astype(f)
    w2s = (w2 * sm2[:, None]).astype(f)            # (128, 256)
    w2maT = np.ascontiguousarray(w2s[:, :128].T.astype(h))  # (128, 128)
    w2mbT = np.ascontiguousarray(w2s[:, 128:].T.astype(h))
    w3T = np.ascontiguousarray(w3.T.astype(h))     # (128, 1)
    choff = np.broadcast_to(
        (np.repeat(np.arange(NCH, dtype=np.uint32), 8) * CHW)[None, :],
        (128, NCH * 8))
    return {
        "wnT": wnT, "wcnT": wcnT, "t1": t1.reshape(C, 1),
        "w2T": w2T, "t2": t2.reshape(C, 1),
        "w1aT": w1aT, "w1bT": w1bT,
        "tm1a": tm1[:128].reshape(128, 1), "tm1b": tm1[128:].reshape(128, 1),
        "w2maT": w2maT, "w2mbT": w2mbT, "tm2": tm2.reshape(128, 1),
        "w3T": w3T, "b3": b3.reshape(1, 1).astype(f),
        "choff": np.ascontiguousarray(choff),
        "negrow": np.full((1, N_FULL), -1.0, dtype=f),
    }


def kernel(**inputs):
    from concourse.bass_utils import run_bass_kernel_spmd

    x = np.asarray(inputs["x"], dtype=np.float32)  # (B, C, N)
    B = x.shape[0]
    n = x.shape[2]
    w = host_weights(**{k: np.asarray(v, dtype=np.float32)
                        for k, v in inputs.items() if k != "x"})
    if n not in _cache:
        _cache[n] = build_nc(n)
    nc = _cache[n]
    in_maps = [{"x": np.ascontiguousarray(x[c]), **w} for c in range(B)]
    res = run_bass_kernel_spmd(nc, in_maps, list(range(NCORES)))
    out = np.stack([res.results[c]["out"][0] for c in range(B)], axis=0)
    return out.astype(np.float32)


# revision 17
# speedup vs baseline: 3.4271x; 1.1609x over previous
"""DGCNN prediction head on 8 Trainium2 NeuronCores.

Data-parallel over batch B=8: each core runs the full pipeline for one
sample (C=64 channels, N=4096 points, k=20 neighbors).

Per-core pipeline (all on one NeuronCore, no collectives):
  1. pairwise ranking R[i,j] = 2<x_i,x_j> - ||x_j||^2 via PE matmul with an
     augmented contract row; R stays in PSUM (no drain).
  2. top-20 per row, chunked: per 512-col chunk DVE max8 + max_index read
     the PSUM tile directly (2 passes over the row total instead of 8).
     The 64 chunk candidates are packed (value mantissa | column index) so
     a 3-round max8/match_replace on the 64-wide union yields the top-20
     indices with no further full-row scans.
  3. EdgeConv1 is linear before the LReLU, so it is precomputed per point:
       conv1(i,j) = Wn x_j + (Wc - Wn) x_i  with BN1 folded in
     A' = s1*(Wn x)        -> fp16 DRAM table, 20 rows gathered per block
                              in ONE indirect DMA
     B' = s1*((Wc-Wn) x)+t1-> fp16 on-chip, broadcast-added per query block
  4. e1 = lrelu(A'_j + B'_i): add on DVE (fp16 2x), lrelu on GPSIMD;
     PE-transpose to channel-major (fp16); EdgeConv2 as fp16 64x64 matmuls
     (BN2 scale folded into W2, bias t2 added during the PSUM drain);
     max over k as fp16 DVE tree; lrelu (monotone, s2 >= 0).
  5. point MLP 64->256->128->1, fp16 weights, BN scales folded, biases
     added during PSUM drains, lrelu on GPSIMD.
"""

import numpy as np

C = 64
K = 20
NEG = 0.2
EPS = 1e-5
NCORES = 8
N_FULL = 4096
NEG_FILL = -3.0e38
NCH = 8          # 512-col chunks per 128-row block
CHW = 512        # chunk width

_cache = {}


def build_nc(n):
    from contextlib import ExitStack

    import concourse.bass as bass
    import concourse.bacc as bacc
    import concourse.mybir as mybir
    import concourse.tile as tile
    from concourse.masks import make_identity

    f32 = mybir.dt.float32
    f32r = mybir.dt.float32r
    f16 = mybir.dt.float16
    u32 = mybir.dt.uint32
    AF = mybir.ActivationFunctionType
    OP = mybir.AluOpType

    nblk = n // 128
    nchk = n // 512

    nc = bacc.Bacc("TRN2", target_bir_lowering=False, debug=False,
                   num_devices=NCORES)

    x_d = nc.dram_tensor("x", [C, n], f32, kind="ExternalInput")
    wnT_d = nc.dram_tensor("wnT", [C, C], f32, kind="ExternalInput")
    wcnT_d = nc.dram_tensor("wcnT", [C, C], f32, kind="ExternalInput")
    t1_d = nc.dram_tensor("t1", [C, 1], f32, kind="ExternalInput")
    w2T_d = nc.dram_tensor("w2T", [C, C], f16, kind="ExternalInput")
    t2_d = nc.dram_tensor("t2", [C, 1], f32, kind="ExternalInput")
    w1aT_d = nc.dram_tensor("w1aT", [C, 128], f16, kind="ExternalInput")
    w1bT_d = nc.dram_tensor("w1bT", [C, 128], f16, kind="ExternalInput")
    tm1a_d = nc.dram_tensor("tm1a", [128, 1], f32, kind="ExternalInput")
    tm1b_d = nc.dram_tensor("tm1b", [128, 1], f32, kind="ExternalInput")
    w2maT_d = nc.dram_tensor("w2maT", [128, 128], f16, kind="ExternalInput")
    w2mbT_d = nc.dram_tensor("w2mbT", [128, 128], f16, kind="ExternalInput")
    tm2_d = nc.dram_tensor("tm2", [128, 1], f32, kind="ExternalInput")
    w3T_d = nc.dram_tensor("w3T", [128, 1], f16, kind="ExternalInput")
    b3_d = nc.dram_tensor("b3", [1, 1], f32, kind="ExternalInput")
    choff_d = nc.dram_tensor("choff", [128, NCH * 8], u32, kind="ExternalInput")
    out_d = nc.dram_tensor("out", [1, n], f32, kind="ExternalOutput")

    with tile.TileContext(nc) as tc, ExitStack() as top:
        cpool = top.enter_context(tc.tile_pool(name="consts", bufs=1))
        dpool = top.enter_context(tc.tile_pool(name="dram", bufs=1, space="DRAM"))
        xpool = top.enter_context(tc.tile_pool(name="xaug", bufs=1))
        hpool = top.enter_context(tc.tile_pool(name="hout", bufs=1))

        # --- constants / weights ---
        ident = cpool.tile([128, 128], f32, tag="ident")
        make_identity(nc, ident[:])
        ident16 = cpool.tile([128, 128], f16, tag="ident16")
        nc.scalar.copy(out=ident16[:], in_=ident[:])
        ones64 = cpool.tile([C, 1], f32, tag="ones64")
        nc.vector.memset(ones64[:], 1.0)

        def load_const(dram, shape, tag, dt=f32):
            t = cpool.tile(shape, dt, tag=tag)
            nc.sync.dma_start(t[:], dram[:])
            return t

        wnT = load_const(wnT_d, [C, C], "wnT")
        wcnT = load_const(wcnT_d, [C, C], "wcnT")
        t1 = load_const(t1_d, [C, 1], "t1")
        w2T = load_const(w2T_d, [C, C], "w2T", f16)
        t2 = load_const(t2_d, [C, 1], "t2")
        w1aT = load_const(w1aT_d, [C, 128], "w1aT", f16)
        w1bT = load_const(w1bT_d, [C, 128], "w1bT", f16)
        tm1a = load_const(tm1a_d, [128, 1], "tm1a")
        tm1b = load_const(tm1b_d, [128, 1], "tm1b")
        w2maT = load_const(w2maT_d, [128, 128], "w2maT", f16)
        w2mbT = load_const(w2mbT_d, [128, 128], "w2mbT", f16)
        tm2 = load_const(tm2_d, [128, 1], "tm2")
        w3T = load_const(w3T_d, [128, 1], "w3T", f16)
        b3 = load_const(b3_d, [1, 1], "b3")
        choff = load_const(choff_d, [128, NCH * 8], "choff", u32)

        At = dpool.tile([n, C], f16, tag="At")           # A' fp16 gather table
        xaug = xpool.tile([C + 1, n], f32, tag="xaug")   # rows 0..63 = x, row 64 = ||x_j||^2
        x2aug = xpool.tile([C + 1, n], f32, tag="x2aug") # rows 0..63 = 2x, row 64 = -1
        Bt = xpool.tile([128, C * nblk], f16, tag="Bt")  # B' fp16, block j at cols 64j
        H = hpool.tile([C, n], f16, tag="H")             # per-point features after edge max
        osb = hpool.tile([1, n], f32, tag="osb")

        # ---------------- stage 0: tables ----------------
        with tc.tile_pool(name="s0sb", bufs=2) as s0sb, \
             tc.tile_pool(name="s0ps", bufs=3, space="PSUM") as s0ps, \
             tc.tile_pool(name="s0pt", bufs=2, space="PSUM") as s0pt:
            # x load split in quarters across queues so chunk 0 lands early
            nq = n // 4
            nc.sync.dma_start(xaug[:C, 0 * nq:1 * nq], x_d[:, 0 * nq:1 * nq])
            nc.scalar.dma_start(xaug[:C, 1 * nq:2 * nq], x_d[:, 1 * nq:2 * nq])
            nc.gpsimd.dma_start(xaug[:C, 2 * nq:3 * nq], x_d[:, 2 * nq:3 * nq])
            nc.sync.dma_start(xaug[:C, 3 * nq:4 * nq], x_d[:, 3 * nq:4 * nq])
            nc.gpsimd.memset(x2aug[C:C + 1, :], -1.0)
            for ch in range(nchk):
                cs = slice(512 * ch, 512 * (ch + 1))
                # 2x copy per chunk (Act), squares on DVE, drains split
                nc.scalar.activation(out=x2aug[:C, cs], in_=xaug[:C, cs],
                                     func=AF.Copy, scale=2.0)
                xsq = s0sb.tile([C, 512], f32, tag="xsq")
                nc.vector.tensor_mul(out=xsq[:], in0=xaug[:C, cs],
                                     in1=xaug[:C, cs])
                psxx = s0ps.tile([1, 512], f32, tag="s0p", space="PSUM")
                nc.tensor.matmul(out=psxx[:], lhsT=ones64[:].bitcast(f32r),
                                 rhs=xsq[:].bitcast(f32r),
                                 start=True, stop=True)
                nc.gpsimd.tensor_copy(out=xaug[C:C + 1, cs], in_=psxx[:])
                psa = s0ps.tile([C, 512], f32, tag="s0p", space="PSUM")
                nc.tensor.matmul(out=psa[:], lhsT=wnT[:].bitcast(f32r),
                                 rhs=xaug[:C, cs].bitcast(f32r),
                                 start=True, stop=True)
                ap = s0sb.tile([C, 512], f16, tag="ap")
                nc.scalar.copy(out=ap[:], in_=psa[:])
                psb = s0ps.tile([C, 512], f32, tag="s0p", space="PSUM")
                nc.tensor.matmul(out=psb[:], lhsT=wcnT[:].bitcast(f32r),
                                 rhs=xaug[:C, cs].bitcast(f32r),
                                 start=True, stop=True)
                bp = s0sb.tile([C, 512], f16, tag="bp")
                nc.vector.tensor_scalar(out=bp[:], in0=psb[:], scalar1=t1[:],
                                        scalar2=None, op0=OP.add)
                ast = s0sb.tile([128, 4 * C], f16, tag="ast")
                for j in range(4):
                    blk = 4 * ch + j
                    js = slice(128 * j, 128 * (j + 1))
                    pta = s0pt.tile([128, C], f16, tag="s0t", space="PSUM")
                    nc.tensor.transpose(out=pta[:], in_=ap[:, js],
                                        identity=ident16[:C, :C])
                    nc.scalar.copy(out=ast[:, C * j:C * (j + 1)], in_=pta[:])
                    ptb = s0pt.tile([128, C], f16, tag="s0t", space="PSUM")
                    nc.tensor.transpose(out=ptb[:], in_=bp[:, js],
                                        identity=ident16[:C, :C])
                    nc.scalar.copy(out=Bt[:, C * blk:C * (blk + 1)], in_=ptb[:])
                nc.sync.dma_start(
                    At[512 * ch:512 * (ch + 1), :].rearrange(
                        "(j p) c -> p j c", p=128),
                    ast[:].rearrange("p (j c) -> p j c", j=4))

        # ---------------- stage 1: blocks ----------------
        with tc.tile_pool(name="cpoolv", bufs=2) as cvp, \
             tc.tile_pool(name="gpool", bufs=2) as gpool, \
             tc.tile_pool(name="epool", bufs=2) as epool, \
             tc.tile_pool(name="wpool", bufs=2) as wpool, \
             tc.tile_pool(name="tpool", bufs=2) as tpool, \
             tc.tile_pool(name="mlpsb", bufs=2) as mlpsb, \
             tc.tile_pool(name="psR", bufs=2, space="PSUM") as psR, \
             tc.tile_pool(name="psT", bufs=2, space="PSUM") as psT, \
             tc.tile_pool(name="psE", bufs=2, space="PSUM") as psE, \
             tc.tile_pool(name="mlpps", bufs=2, space="PSUM") as mlpps:

            def mlp_chunk(ch):
                cs = slice(512 * ch, 512 * (ch + 1))
                l1a = mlpsb.tile([128, 512], f16, tag="l1a")
                l1b = mlpsb.tile([128, 512], f16, tag="l1b")
                ps1a = mlpps.tile([128, 512], f32, tag="mlpp", space="PSUM")
                nc.tensor.matmul(out=ps1a[:], lhsT=w1aT[:], rhs=H[:, cs],
                                 start=True, stop=True)
                nc.scalar.activation(out=l1a[:], in_=ps1a[:],
                                     func=AF.Identity, bias=tm1a[:], scale=1.0)
                nc.gpsimd.scalar_tensor_tensor(
                    out=l1a[:], in0=l1a[:], scalar=NEG, in1=l1a[:],
                    op0=OP.mult, op1=OP.max)
                ps1b = mlpps.tile([128, 512], f32, tag="mlpp", space="PSUM")
                nc.tensor.matmul(out=ps1b[:], lhsT=w1bT[:], rhs=H[:, cs],
                                 start=True, stop=True)
                nc.scalar.activation(out=l1b[:], in_=ps1b[:],
                                     func=AF.Identity, bias=tm1b[:], scale=1.0)
                nc.gpsimd.scalar_tensor_tensor(
                    out=l1b[:], in0=l1b[:], scalar=NEG, in1=l1b[:],
                    op0=OP.mult, op1=OP.max)
                ps2 = mlpps.tile([128, 512], f32, tag="mlpp", space="PSUM")
                nc.tensor.matmul(out=ps2[:], lhsT=w2maT[:], rhs=l1a[:],
                                 start=True, stop=False)
                nc.tensor.matmul(out=ps2[:], lhsT=w2mbT[:], rhs=l1b[:],
                                 start=False, stop=True)
                l2 = mlpsb.tile([128, 512], f16, tag="l2")
                nc.scalar.activation(out=l2[:], in_=ps2[:],
                                     func=AF.Identity, bias=tm2[:], scale=1.0)
                nc.gpsimd.scalar_tensor_tensor(
                    out=l2[:], in0=l2[:], scalar=NEG, in1=l2[:],
                    op0=OP.mult, op1=OP.max)
                ps3 = mlpps.tile([1, 512], f32, tag="mlpp", space="PSUM")
                nc.tensor.matmul(out=ps3[:], lhsT=w3T[:], rhs=l2[:],
                                 start=True, stop=True)
                nc.scalar.activation(out=osb[:, cs], in_=ps3[:],
                                     func=AF.Identity, bias=b3[:], scale=1.0)

            for b in range(nblk):
                bs = slice(128 * b, 128 * (b + 1))

                # pairwise + chunked top-8, straight from PSUM
                cand_v = cvp.tile([128, NCH * 8], f32, tag="cv")
                cand_i = cvp.tile([128, NCH * 8], u32, tag="ci")
                for ch in range(NCH):
                    cs = slice(CHW * ch, CHW * (ch + 1))
                    ks = slice(8 * ch, 8 * (ch + 1))
                    ps = psR.tile([128, CHW], f32, tag="psr", space="PSUM")
                    nc.tensor.matmul(out=ps[:], lhsT=x2aug[:, bs].bitcast(f32r),
                                     rhs=xaug[:, cs].bitcast(f32r),
                                     start=True, stop=True)
                    nc.vector.max(out=cand_v[:, ks], in_=ps[:])
                    nc.vector.max_index(out=cand_i[:, ks],
                                        in_max=cand_v[:, ks], in_values=ps[:])

                # pack value|index on GPSIMD, union top-20 on DVE
                pk = cvp.tile([128, NCH * 8], u32, tag="pk")
                nc.gpsimd.tensor_tensor(out=cand_i[:], in0=cand_i[:],
                                        in1=choff[:], op=OP.add)
                nc.gpsimd.tensor_scalar(out=pk[:], in0=cand_v[:].bitcast(u32),
                                        scalar1=0xFFFFF000, scalar2=None,
                                        op0=OP.bitwise_and)
                nc.gpsimd.tensor_tensor(out=pk[:], in0=pk[:], in1=cand_i[:],
                                        op=OP.bitwise_or)
                pkf = pk[:].bitcast(f32)
                pv1 = cvp.tile([128, 8], f32, tag="pv1")
                pv2 = cvp.tile([128, 8], f32, tag="pv2")
                pv3 = cvp.tile([128, 8], f32, tag="pv3")
                nc.vector.max(out=pv1[:], in_=pkf)
                nc.vector.match_replace(out=pkf, in_to_replace=pv1[:],
                                        in_values=pkf, imm_value=NEG_FILL)
                nc.vector.max(out=pv2[:], in_=pkf)
                nc.vector.match_replace(out=pkf, in_to_replace=pv2[:],
                                        in_values=pkf, imm_value=NEG_FILL)
                nc.vector.max(out=pv3[:], in_=pkf)
                idx = cvp.tile([128, 24], u32, tag="idx")
                nc.gpsimd.tensor_scalar(out=idx[:, 0:8], in0=pv1[:].bitcast(u32),
                                        scalar1=0xFFF, scalar2=None,
                                        op0=OP.bitwise_and)
                nc.gpsimd.tensor_scalar(out=idx[:, 8:16], in0=pv2[:].bitcast(u32),
                                        scalar1=0xFFF, scalar2=None,
                                        op0=OP.bitwise_and)
                nc.gpsimd.tensor_scalar(out=idx[:, 16:24], in0=pv3[:].bitcast(u32),
                                        scalar1=0xFFF, scalar2=None,
                                        op0=OP.bitwise_and)

                # gather all 20 neighbors in one indirect DMA (fp16 rows)
                G = gpool.tile([128, K * C], f16, tag="G")
                nc.gpsimd.indirect_dma_start(
                    out=G[:].rearrange("p (k c) -> p k c", k=K),
                    out_offset=None,
                    in_=At[:],
                    in_offset=bass.IndirectOffsetOnAxis(ap=idx[:, 0:K], axis=0))

                # e1 = lrelu(G + B'_i): add + lrelu both on GPSIMD
                bb = Bt[:, C * b:C * (b + 1)].rearrange(
                    "p (k c) -> p k c", k=1).to_broadcast([128, K, C])
                nc.gpsimd.tensor_tensor(
                    out=G[:].rearrange("p (k c) -> p k c", k=K),
                    in0=G[:].rearrange("p (k c) -> p k c", k=K),
                    in1=bb, op=OP.add)
                nc.gpsimd.scalar_tensor_tensor(
                    out=G[:], in0=G[:], scalar=NEG, in1=G[:],
                    op0=OP.mult, op1=OP.max)

                # transpose to channel-major: 20 PE transposes [128,64]->[64,128]
                e1T = epool.tile([C, K * 128], f16, tag="e1T")
                for grp, gw in ((0, 8), (1, 8), (2, 4)):
                    pt = psT.tile([C, 1024], f16, tag="pst", space="PSUM")
                    for s in range(gw):
                        k = 8 * grp + s
                        nc.tensor.transpose(
                            out=pt[:, 128 * s:128 * (s + 1)],
                            in_=G[:, C * k:C * (k + 1)],
                            identity=ident16[:])
                    nc.scalar.copy(
                        out=e1T[:, 1024 * grp:1024 * grp + 128 * gw],
                        in_=pt[:, :128 * gw])

                # conv2 (w_k2 with bn2 scale folded), t2 added in drain
                ew = wpool.tile([C, K * 128], f16, tag="ew")
                for grp in range(5):
                    pe = psE.tile([C, 512], f32, tag="pse", space="PSUM")
                    for s in range(4):
                        k = 4 * grp + s
                        nc.tensor.matmul(
                            out=pe[:, 128 * s:128 * (s + 1)],
                            lhsT=w2T[:],
                            rhs=e1T[:, 128 * k:128 * (k + 1)],
                            start=True, stop=True)
                    nc.scalar.activation(
                        out=ew[:, 512 * grp:512 * (grp + 1)], in_=pe[:],
                        func=AF.Identity, bias=t2[:], scale=1.0)

                # max over k: levels 1-2 on GPSIMD, rest + lrelu on DVE
                m1 = tpool.tile([C, 10 * 128], f16, tag="m1")
                nc.gpsimd.tensor_tensor(out=m1[:], in0=ew[:, :1280],
                                        in1=ew[:, 1280:], op=OP.max)
                m2 = tpool.tile([C, 5 * 128], f16, tag="m2")
                nc.gpsimd.tensor_tensor(out=m2[:], in0=m1[:, :640],
                                        in1=m1[:, 640:], op=OP.max)
                m3 = tpool.tile([C, 2 * 128], f16, tag="m3")
                nc.gpsimd.tensor_tensor(out=m3[:], in0=m2[:, :256],
                                        in1=m2[:, 256:512], op=OP.max)
                m4 = tpool.tile([C, 128], f16, tag="m4")
                nc.gpsimd.tensor_tensor(out=m4[:], in0=m3[:, :128],
                                        in1=m3[:, 128:], op=OP.max)
                nc.gpsimd.tensor_tensor(out=m4[:], in0=m4[:],
                                        in1=m2[:, 512:], op=OP.max)
                nc.gpsimd.scalar_tensor_tensor(
                    out=H[:, bs], in0=m4[:], scalar=NEG, in1=m4[:],
                    op0=OP.mult, op1=OP.max)

                # point MLP for the finished 512-col chunk, interleaved
                if b % 4 == 3:
                    mlp_chunk(b // 4)

            nc.sync.dma_start(out_d[:], osb[:])

    nc.finalize()
    return nc


def host_weights(w_k1, g_k1, b_k1, m_k1, v_k1, w_k2, g_k2, b_k2, m_k2, v_k2,
                 w1, g1, b1, m1, v1, w2, g2, b2, m2, v2, w3, b3):
    f = np.float32
    h = np.float16
    s1 = (g_k1 / np.sqrt(v_k1 + f(EPS))).astype(f)
    t1 = (b_k1 - m_k1 * s1).astype(f)
    wn = w_k1[:, :C]
    wc = w_k1[:, C:]
    wnT = np.ascontiguousarray((wn * s1[:, None]).T.astype(f))
    wcnT = np.ascontiguousarray(((wc - wn) * s1[:, None]).T.astype(f))
    s2 = (g_k2 / np.sqrt(v_k2 + f(EPS))).astype(f)
    t2 = (b_k2 - m_k2 * s2).astype(f)
    w2T = np.ascontiguousarray((w_k2 * s2[:, None]).T.astype(h))
    sm1 = (g1 / np.sqrt(v1 + f(EPS))).astype(f)
    tm1 = (b1 - m1 * sm1).astype(f)
    w1s = (w1 * sm1[:, None]).astype(f)            # (256, 64)
    w1aT = np.ascontiguousarray(w1s[:128].T.astype(h))  # (64, 128)
    w1bT = np.ascontiguousarray(w1s[128:].T.astype(h))
    sm2 = (g2 / np.sqrt(v2 + f(EPS))).astype(f)
    tm2 = (b2 - m2 * sm2).astype(f)
    w2s = (w2 * sm2[:, None]).astype(f)            # (128, 256)
    w2maT = np.ascontiguousarray(w2s[:, :128].T.astype(h))  # (128, 128)
    w2mbT = np.ascontiguousarray(w2s[:, 128:].T.astype(h))
    w3T = np.ascontiguousarray(w3.T.astype(h))     # (128, 1)
    choff = np.broadcast_to(
        (np.repeat(np.arange(NCH, dtype=np.uint32), 8) * CHW)[None, :],
        (128, NCH * 8))
    return {
        "wnT": wnT, "wcnT": wcnT, "t1": t1.reshape(C, 1),
        "w2T": w2T, "t2": t2.reshape(C, 1),
        "w1aT": w1aT, "w1bT": w1bT,
        "tm1a": tm1[:128].reshape(128, 1), "tm1b": tm1[128:].reshape(128, 1),
        "w2maT": w2maT, "w2mbT": w2mbT, "tm2": tm2.reshape(128, 1),
        "w3T": w3T, "b3": b3.reshape(1, 1).astype(f),
        "choff": np.ascontiguousarray(choff),
    }


def kernel(**inputs):
    from concourse.bass_utils import run_bass_kernel_spmd

    x = np.asarray(inputs["x"], dtype=np.float32)  # (B, C, N)
    B = x.shape[0]
    n = x.shape[2]
    w = host_weights(**{k: np.asarray(v, dtype=np.float32)
                        for k, v in inputs.items() if k != "x"})
    if n not in _cache:
        _cache[n] = build_nc(n)
    nc = _cache[n]
    in_maps = [{"x": np.ascontiguousarray(x[c]), **w} for c in range(B)]
    res = run_bass_kernel_spmd(nc, in_maps, list(range(NCORES)))
    out = np.stack([res.results[c]["out"][0] for c in range(B)], axis=0)
    return out.astype(np.float32)


# revision 20
# speedup vs baseline: 3.4743x; 1.0138x over previous
"""DGCNN prediction head on 8 Trainium2 NeuronCores.

Data-parallel over batch B=8: each core runs the full pipeline for one
sample (C=64 channels, N=4096 points, k=20 neighbors).

Per-core pipeline (all on one NeuronCore, no collectives):
  1. pairwise ranking R[i,j] = 2<x_i,x_j> - ||x_j||^2 via PE matmul with an
     augmented contract row; R stays in PSUM (no drain).
  2. top-20 per row, chunked: per 512-col chunk DVE max8 + max_index read
     the PSUM tile directly (2 passes over the row total instead of 8).
     The 64 chunk candidates are packed (value mantissa | column index) so
     a 3-round max8/match_replace on the 64-wide union yields the top-20
     indices with no further full-row scans.
  3. EdgeConv1 is linear before the LReLU, so it is precomputed per point:
       conv1(i,j) = Wn x_j + (Wc - Wn) x_i  with BN1 folded in
     A' = s1*(Wn x)        -> fp16 DRAM table, 20 rows gathered per block
                              in ONE indirect DMA
     B' = s1*((Wc-Wn) x)+t1-> fp16 on-chip, broadcast-added per query block
  4. e1 = lrelu(A'_j + B'_i): add on DVE (fp16 2x), lrelu on GPSIMD;
     PE-transpose to channel-major (fp16); EdgeConv2 as fp16 64x64 matmuls
     (BN2 scale folded into W2, bias t2 added during the PSUM drain);
     max over k as fp16 DVE tree; lrelu (monotone, s2 >= 0).
  5. point MLP 64->256->128->1, fp16 weights, BN scales folded, biases
     added during PSUM drains, lrelu on GPSIMD.
"""

import numpy as np

C = 64
K = 20
NEG = 0.2
EPS = 1e-5
NCORES = 8
N_FULL = 4096
NEG_FILL = -3.0e38
NCH = 8          # 512-col chunks per 128-row block
CHW = 512        # chunk width

_cache = {}


def build_nc(n):
    from contextlib import ExitStack

    import concourse.bass as bass
    import concourse.bacc as bacc
    import concourse.mybir as mybir
    import concourse.tile as tile
    from concourse.masks import make_identity

    f32 = mybir.dt.float32
    f32r = mybir.dt.float32r
    f16 = mybir.dt.float16
    u32 = mybir.dt.uint32
    AF = mybir.ActivationFunctionType
    OP = mybir.AluOpType

    nblk = n // 128
    nchk = n // 512

    nc = bacc.Bacc("TRN2", target_bir_lowering=False, debug=False,
                   num_devices=NCORES)

    x_d = nc.dram_tensor("x", [C, n], f32, kind="ExternalInput")
    wnT_d = nc.dram_tensor("wnT", [C, C], f32, kind="ExternalInput")
    wcnT_d = nc.dram_tensor("wcnT", [C, C], f32, kind="ExternalInput")
    t1_d = nc.dram_tensor("t1", [C, 1], f32, kind="ExternalInput")
    w2T_d = nc.dram_tensor("w2T", [C, C], f16, kind="ExternalInput")
    t2_d = nc.dram_tensor("t2", [C, 1], f32, kind="ExternalInput")
    w1aT_d = nc.dram_tensor("w1aT", [C, 128], f16, kind="ExternalInput")
    w1bT_d = nc.dram_tensor("w1bT", [C, 128], f16, kind="ExternalInput")
    tm1a_d = nc.dram_tensor("tm1a", [128, 1], f32, kind="ExternalInput")
    tm1b_d = nc.dram_tensor("tm1b", [128, 1], f32, kind="ExternalInput")
    w2maT_d = nc.dram_tensor("w2maT", [128, 128], f16, kind="ExternalInput")
    w2mbT_d = nc.dram_tensor("w2mbT", [128, 128], f16, kind="ExternalInput")
    tm2_d = nc.dram_tensor("tm2", [128, 1], f32, kind="ExternalInput")
    w3T_d = nc.dram_tensor("w3T", [128, 1], f16, kind="ExternalInput")
    b3_d = nc.dram_tensor("b3", [1, 1], f32, kind="ExternalInput")
    choff_d = nc.dram_tensor("choff", [128, NCH * 8], u32, kind="ExternalInput")
    out_d = nc.dram_tensor("out", [1, n], f32, kind="ExternalOutput")

    with tile.TileContext(nc) as tc, ExitStack() as top:
        cpool = top.enter_context(tc.tile_pool(name="consts", bufs=1))
        dpool = top.enter_context(tc.tile_pool(name="dram", bufs=1, space="DRAM"))
        xpool = top.enter_context(tc.tile_pool(name="xaug", bufs=1))
        hpool = top.enter_context(tc.tile_pool(name="hout", bufs=1))

        # --- constants / weights ---
        ident = cpool.tile([128, 128], f32, tag="ident")
        make_identity(nc, ident[:])
        ident16 = cpool.tile([128, 128], f16, tag="ident16")
        nc.scalar.copy(out=ident16[:], in_=ident[:])
        ones64 = cpool.tile([C, 1], f32, tag="ones64")
        nc.vector.memset(ones64[:], 1.0)

        def load_const(dram, shape, tag, dt=f32):
            t = cpool.tile(shape, dt, tag=tag)
            nc.sync.dma_start(t[:], dram[:])
            return t

        wnT = load_const(wnT_d, [C, C], "wnT")
        wcnT = load_const(wcnT_d, [C, C], "wcnT")
        t1 = load_const(t1_d, [C, 1], "t1")
        w2T = load_const(w2T_d, [C, C], "w2T", f16)
        t2 = load_const(t2_d, [C, 1], "t2")
        w1aT = load_const(w1aT_d, [C, 128], "w1aT", f16)
        w1bT = load_const(w1bT_d, [C, 128], "w1bT", f16)
        tm1a = load_const(tm1a_d, [128, 1], "tm1a")
        tm1b = load_const(tm1b_d, [128, 1], "tm1b")
        w2maT = load_const(w2maT_d, [128, 128], "w2maT", f16)
        w2mbT = load_const(w2mbT_d, [128, 128], "w2mbT", f16)
        tm2 = load_const(tm2_d, [128, 1], "tm2")
        w3T = load_const(w3T_d, [128, 1], "w3T", f16)
        b3 = load_const(b3_d, [1, 1], "b3")
        choff = load_const(choff_d, [128, NCH * 8], "choff", u32)

        At = dpool.tile([n, C], f16, tag="At")           # A' fp16 gather table
        xaug = xpool.tile([C + 1, n], f32, tag="xaug")   # rows 0..63 = x, row 64 = ||x_j||^2
        x2aug = xpool.tile([C + 1, n], f32, tag="x2aug") # rows 0..63 = 2x, row 64 = -1
        Bt = xpool.tile([128, C * nblk], f16, tag="Bt")  # B' fp16, block j at cols 64j
        H = hpool.tile([C, n], f16, tag="H")             # per-point features after edge max
        osb = hpool.tile([1, n], f32, tag="osb")

        # ---------------- stage 0: tables ----------------
        with tc.tile_pool(name="s0sb", bufs=2) as s0sb, \
             tc.tile_pool(name="s0ps", bufs=3, space="PSUM") as s0ps, \
             tc.tile_pool(name="s0pt", bufs=2, space="PSUM") as s0pt:
            # x load split in quarters across queues so chunk 0 lands early
            nq = n // 4
            nc.sync.dma_start(xaug[:C, 0 * nq:1 * nq], x_d[:, 0 * nq:1 * nq])
            nc.scalar.dma_start(xaug[:C, 1 * nq:2 * nq], x_d[:, 1 * nq:2 * nq])
            nc.gpsimd.dma_start(xaug[:C, 2 * nq:3 * nq], x_d[:, 2 * nq:3 * nq])
            nc.sync.dma_start(xaug[:C, 3 * nq:4 * nq], x_d[:, 3 * nq:4 * nq])
            nc.gpsimd.memset(x2aug[C:C + 1, :], -1.0)
            for ch in range(nchk):
                cs = slice(512 * ch, 512 * (ch + 1))
                # 2x copy per chunk (Act), squares on DVE, drains split
                nc.scalar.activation(out=x2aug[:C, cs], in_=xaug[:C, cs],
                                     func=AF.Copy, scale=2.0)
                xsq = s0sb.tile([C, 512], f32, tag="xsq")
                nc.vector.tensor_mul(out=xsq[:], in0=xaug[:C, cs],
                                     in1=xaug[:C, cs])
                psxx = s0ps.tile([1, 512], f32, tag="s0p", space="PSUM")
                nc.tensor.matmul(out=psxx[:], lhsT=ones64[:].bitcast(f32r),
                                 rhs=xsq[:].bitcast(f32r),
                                 start=True, stop=True)
                nc.gpsimd.tensor_copy(out=xaug[C:C + 1, cs], in_=psxx[:])
                psa = s0ps.tile([C, 512], f32, tag="s0p", space="PSUM")
                nc.tensor.matmul(out=psa[:], lhsT=wnT[:].bitcast(f32r),
                                 rhs=xaug[:C, cs].bitcast(f32r),
                                 start=True, stop=True)
                ap = s0sb.tile([C, 512], f16, tag="ap")
                nc.gpsimd.tensor_copy(out=ap[:], in_=psa[:])
                psb = s0ps.tile([C, 512], f32, tag="s0p", space="PSUM")
                nc.tensor.matmul(out=psb[:], lhsT=wcnT[:].bitcast(f32r),
                                 rhs=xaug[:C, cs].bitcast(f32r),
                                 start=True, stop=True)
                bp = s0sb.tile([C, 512], f16, tag="bp")
                nc.vector.tensor_scalar(out=bp[:], in0=psb[:], scalar1=t1[:],
                                        scalar2=None, op0=OP.add)
                ast = s0sb.tile([128, 4 * C], f16, tag="ast")
                for j in range(4):
                    blk = 4 * ch + j
                    js = slice(128 * j, 128 * (j + 1))
                    pta = s0pt.tile([128, C], f16, tag="s0t", space="PSUM")
                    nc.tensor.transpose(out=pta[:], in_=ap[:, js],
                                        identity=ident16[:C, :C])
                    nc.gpsimd.tensor_copy(out=ast[:, C * j:C * (j + 1)],
                                          in_=pta[:])
                    ptb = s0pt.tile([128, C], f16, tag="s0t", space="PSUM")
                    nc.tensor.transpose(out=ptb[:], in_=bp[:, js],
                                        identity=ident16[:C, :C])
                    nc.scalar.copy(out=Bt[:, C * blk:C * (blk + 1)], in_=ptb[:])
                nc.sync.dma_start(
                    At[512 * ch:512 * (ch + 1), :].rearrange(
                        "(j p) c -> p j c", p=128),
                    ast[:].rearrange("p (j c) -> p j c", j=4))

        # ---------------- stage 1: blocks ----------------
        with tc.tile_pool(name="cpoolv", bufs=2) as cvp, \
             tc.tile_pool(name="gpool", bufs=2) as gpool, \
             tc.tile_pool(name="epool", bufs=2) as epool, \
             tc.tile_pool(name="wpool", bufs=2) as wpool, \
             tc.tile_pool(name="tpool", bufs=2) as tpool, \
             tc.tile_pool(name="mlpsb", bufs=2) as mlpsb, \
             tc.tile_pool(name="psR", bufs=2, space="PSUM") as psR, \
             tc.tile_pool(name="psT", bufs=2, space="PSUM") as psT, \
             tc.tile_pool(name="psE", bufs=2, space="PSUM") as psE, \
             tc.tile_pool(name="mlpps", bufs=2, space="PSUM") as mlpps:

            def mlp_chunk(ch):
                cs = slice(512 * ch, 512 * (ch + 1))
                l1a = mlpsb.tile([128, 512], f16, tag="l1a")
                l1b = mlpsb.tile([128, 512], f16, tag="l1b")
                ps1a = mlpps.tile([128, 512], f32, tag="mlpp", space="PSUM")
                nc.tensor.matmul(out=ps1a[:], lhsT=w1aT[:], rhs=H[:, cs],
                                 start=True, stop=True)
                nc.scalar.activation(out=l1a[:], in_=ps1a[:],
                                     func=AF.Identity, bias=tm1a[:], scale=1.0)
                nc.gpsimd.scalar_tensor_tensor(
                    out=l1a[:], in0=l1a[:], scalar=NEG, in1=l1a[:],
                    op0=OP.mult, op1=OP.max)
                ps1b = mlpps.tile([128, 512], f32, tag="mlpp", space="PSUM")
                nc.tensor.matmul(out=ps1b[:], lhsT=w1bT[:], rhs=H[:, cs],
                                 start=True, stop=True)
                nc.scalar.activation(out=l1b[:], in_=ps1b[:],
                                     func=AF.Identity, bias=tm1b[:], scale=1.0)
                nc.gpsimd.scalar_tensor_tensor(
                    out=l1b[:], in0=l1b[:], scalar=NEG, in1=l1b[:],
                    op0=OP.mult, op1=OP.max)
                ps2 = mlpps.tile([128, 512], f32, tag="mlpp", space="PSUM")
                nc.tensor.matmul(out=ps2[:], lhsT=w2maT[:], rhs=l1a[:],
                                 start=True, stop=False)
                nc.tensor.matmul(out=ps2[:], lhsT=w2mbT[:], rhs=l1b[:],
                                 start=False, stop=True)
                l2 = mlpsb.tile([128, 512], f16, tag="l2")
                nc.scalar.activation(out=l2[:], in_=ps2[:],
                                     func=AF.Identity, bias=tm2[:], scale=1.0)
                nc.gpsimd.scalar_tensor_tensor(
                    out=l2[:], in0=l2[:], scalar=NEG, in1=l2[:],
                    op0=OP.mult, op1=OP.max)
                ps3 = mlpps.tile([1, 512], f32, tag="mlpp", space="PSUM")
                nc.tensor.matmul(out=ps3[:], lhsT=w3T[:], rhs=l2[:],
                                 start=True, stop=True)
                nc.scalar.activation(out=osb[:, cs], in_=ps3[:],
                                     func=AF.Identity, bias=b3[:], scale=1.0)

            for b in range(nblk):
                bs = slice(128 * b, 128 * (b + 1))

                # pairwise + chunked top-8, straight from PSUM
                cand_v = cvp.tile([128, NCH * 8], f32, tag="cv")
                cand_i = cvp.tile([128, NCH * 8], u32, tag="ci")
                for ch in range(NCH):
                    cs = slice(CHW * ch, CHW * (ch + 1))
                    ks = slice(8 * ch, 8 * (ch + 1))
                    ps = psR.tile([128, CHW], f32, tag="psr", space="PSUM")
                    nc.tensor.matmul(out=ps[:], lhsT=x2aug[:, bs].bitcast(f32r),
                                     rhs=xaug[:, cs].bitcast(f32r),
                                     start=True, stop=True)
                    nc.vector.max(out=cand_v[:, ks], in_=ps[:])
                    nc.vector.max_index(out=cand_i[:, ks],
                                        in_max=cand_v[:, ks], in_values=ps[:])

                # pack value|index on GPSIMD, union top-20 on DVE
                pk = cvp.tile([128, NCH * 8], u32, tag="pk")
                nc.gpsimd.tensor_tensor(out=cand_i[:], in0=cand_i[:],
                                        in1=choff[:], op=OP.add)
                nc.gpsimd.tensor_scalar(out=pk[:], in0=cand_v[:].bitcast(u32),
                                        scalar1=0xFFFFF000, scalar2=None,
                                        op0=OP.bitwise_and)
                nc.gpsimd.tensor_tensor(out=pk[:], in0=pk[:], in1=cand_i[:],
                                        op=OP.bitwise_or)
                pkf = pk[:].bitcast(f32)
                pv1 = cvp.tile([128, 8], f32, tag="pv1")
                pv2 = cvp.tile([128, 8], f32, tag="pv2")
                pv3 = cvp.tile([128, 8], f32, tag="pv3")
                nc.vector.max(out=pv1[:], in_=pkf)
                nc.vector.match_replace(out=pkf, in_to_replace=pv1[:],
                                        in_values=pkf, imm_value=NEG_FILL)
                nc.vector.max(out=pv2[:], in_=pkf)
                nc.vector.match_replace(out=pkf, in_to_replace=pv2[:],
                                        in_values=pkf, imm_value=NEG_FILL)
                nc.vector.max(out=pv3[:], in_=pkf)
                idx = cvp.tile([128, 24], u32, tag="idx")
                nc.gpsimd.tensor_scalar(out=idx[:, 0:8], in0=pv1[:].bitcast(u32),
                                        scalar1=0xFFF, scalar2=None,
                                        op0=OP.bitwise_and)
                nc.gpsimd.tensor_scalar(out=idx[:, 8:16], in0=pv2[:].bitcast(u32),
                                        scalar1=0xFFF, scalar2=None,
                                        op0=OP.bitwise_and)
                nc.gpsimd.tensor_scalar(out=idx[:, 16:24], in0=pv3[:].bitcast(u32),
                                        scalar1=0xFFF, scalar2=None,
                                        op0=OP.bitwise_and)

                # gather all 20 neighbors in one indirect DMA (fp16 rows)
                G = gpool.tile([128, K * C], f16, tag="G")
                nc.gpsimd.indirect_dma_start(
                    out=G[:].rearrange("p (k c) -> p k c", k=K),
                    out_offset=None,
                    in_=At[:],
                    in_offset=bass.IndirectOffsetOnAxis(ap=idx[:, 0:K], axis=0))

                # e1 = lrelu(G + B'_i): add + lrelu both on GPSIMD
                bb = Bt[:, C * b:C * (b + 1)].rearrange(
                    "p (k c) -> p k c", k=1).to_broadcast([128, K, C])
                nc.gpsimd.tensor_tensor(
                    out=G[:].rearrange("p (k c) -> p k c", k=K),
                    in0=G[:].rearrange("p (k c) -> p k c", k=K),
                    in1=bb, op=OP.add)
                nc.gpsimd.scalar_tensor_tensor(
                    out=G[:], in0=G[:], scalar=NEG, in1=G[:],
                    op0=OP.mult, op1=OP.max)

                # transpose to channel-major: 20 PE transposes [128,64]->[64,128]
                e1T = epool.tile([C, K * 128], f16, tag="e1T")
                for grp, gw in ((0, 8), (1, 8), (2, 4)):
                    pt = psT.tile([C, 1024], f16, tag="pst", space="PSUM")
                    for s in range(gw):
                        k = 8 * grp + s
                        nc.tensor.transpose(
                            out=pt[:, 128 * s:128 * (s + 1)],
                            in_=G[:, C * k:C * (k + 1)],
                            identity=ident16[:])
                    nc.scalar.copy(
                        out=e1T[:, 1024 * grp:1024 * grp + 128 * gw],
                        in_=pt[:, :128 * gw])

                # conv2 (w_k2 with bn2 scale folded), t2 added in drain
                ew = wpool.tile([C, K * 128], f16, tag="ew")
                for grp in range(5):
                    pe = psE.tile([C, 512], f32, tag="pse", space="PSUM")
                    for s in range(4):
                        k = 4 * grp + s
                        nc.tensor.matmul(
                            out=pe[:, 128 * s:128 * (s + 1)],
                            lhsT=w2T[:],
                            rhs=e1T[:, 128 * k:128 * (k + 1)],
                            start=True, stop=True)
                    nc.scalar.activation(
                        out=ew[:, 512 * grp:512 * (grp + 1)], in_=pe[:],
                        func=AF.Identity, bias=t2[:], scale=1.0)

                # max over k: levels 1-2 on GPSIMD, rest + lrelu on DVE
                m1 = tpool.tile([C, 10 * 128], f16, tag="m1")
                nc.gpsimd.tensor_tensor(out=m1[:], in0=ew[:, :1280],
                                        in1=ew[:, 1280:], op=OP.max)
                m2 = tpool.tile([C, 5 * 128], f16, tag="m2")
                nc.gpsimd.tensor_tensor(out=m2[:], in0=m1[:, :640],
                                        in1=m1[:, 640:], op=OP.max)
                m3 = tpool.tile([C, 2 * 128], f16, tag="m3")
                nc.gpsimd.tensor_tensor(out=m3[:], in0=m2[:, :256],
                                        in1=m2[:, 256:512], op=OP.max)
                m4 = tpool.tile([C, 128], f16, tag="m4")
                nc.gpsimd.tensor_tensor(out=m4[:], in0=m3[:, :128],
                                        in1=m3[:, 128:], op=OP.max)
                nc.gpsimd.tensor_tensor(out=m4[:], in0=m4[:],
                                        in1=m2[:, 512:], op=OP.max)
                nc.gpsimd.scalar_tensor_tensor(
                    out=H[:, bs], in0=m4[:], scalar=NEG, in1=m4[:],
                    op0=OP.mult, op1=OP.max)

                # point MLP for the finished 512-col chunk, interleaved
                if b % 4 == 3:
                    mlp_chunk(b // 4)

            nc.sync.dma_start(out_d[:], osb[:])

    nc.finalize()
    return nc


def host_weights(w_k1, g_k1, b_k1, m_k1, v_k1, w_k2, g_k2, b_k2, m_k2, v_k2,
                 w1, g1, b1, m1, v1, w2, g2, b2, m2, v2, w3, b3):
    f = np.float32
    h = np.float16
    s1 = (g_k1 / np.sqrt(v_k1 + f(EPS))).astype(f)
    t1 = (b_k1 - m_k1 * s1).astype(f)
    wn = w_k1[:, :C]
    wc = w_k1[:, C:]
    wnT = np.ascontiguousarray((wn * s1[:, None]).T.astype(f))
    wcnT = np.ascontiguousarray(((wc - wn) * s1[:, None]).T.astype(f))
    s2 = (g_k2 / np.sqrt(v_k2 + f(EPS))).astype(f)
    t2 = (b_k2 - m_k2 * s2).astype(f)
    w2T = np.ascontiguousarray((w_k2 * s2[:, None]).T.astype(h))
    sm1 = (g1 / np.sqrt(v1 + f(EPS))).astype(f)
    tm1 = (b1 - m1 * sm1).astype(f)
    w1s = (w1 * sm1[:, None]).astype(f)            # (256, 64)
    w1aT = np.ascontiguousarray(w1s[:128].T.astype(h))  # (64, 128)
    w1bT = np.ascontiguousarray(w1s[128:].T.astype(h))
    sm2 = (g2 / np.sqrt(v2 + f(EPS))).astype(f)
    tm2 = (b2 - m2 * sm2).astype(f)
    w2s = (w2 * sm2[:, None]).astype(f)            # (128, 256)
    w2maT = np.ascontiguousarray(w2s[:, :128].T.astype(h))  # (128, 128)
    w2mbT = np.ascontiguousarray(w2s[:, 128:].T.astype(h))
    w3T = np.ascontiguousarray(w3.T.astype(h))     # (128, 1)
    choff = np.broadcast_to(
        (np.repeat(np.arange(NCH, dtype=np.uint32), 8) * CHW)[None, :],
        (128, NCH * 8))
    return {
        "wnT": wnT, "wcnT": wcnT, "t1": t1.reshape(C, 1),
        "w2T": w2T, "t2": t2.reshape(C, 1),
        "w1aT": w1aT, "w1bT": w1bT,
        "tm1a": tm1[:128].reshape(128, 1), "tm1b": tm1[128:].reshape(128, 1),
        "w2maT": w2maT, "w2mbT": w2mbT, "tm2": tm2.reshape(128, 1),
        "w3T": w3T, "b3": b3.reshape(1, 1).astype(f),
        "choff": np.ascontiguousarray(choff),
    }


def kernel(**inputs):
    from concourse.bass_utils import run_bass_kernel_spmd

    x = np.asarray(inputs["x"], dtype=np.float32)  # (B, C, N)
    B = x.shape[0]
    n = x.shape[2]
    w = host_weights(**{k: np.asarray(v, dtype=np.float32)
                        for k, v in inputs.items() if k != "x"})
    if n not in _cache:
        _cache[n] = build_nc(n)
    nc = _cache[n]
    in_maps = [{"x": np.ascontiguousarray(x[c]), **w} for c in range(B)]
    res = run_bass_kernel_spmd(nc, in_maps, list(range(NCORES)))
    out = np.stack([res.results[c]["out"][0] for c in range(B)], axis=0)
    return out.astype(np.float32)


# revision 23
# speedup vs baseline: 3.7796x; 1.0879x over previous
"""DGCNN prediction head on 8 Trainium2 NeuronCores.

Data-parallel over batch B=8: each core runs the full pipeline for one
sample (C=64 channels, N=4096 points, k=20 neighbors).

Per-core pipeline (all on one NeuronCore, no collectives):
  1. pairwise ranking R[i,j] = 2<x_i,x_j> - ||x_j||^2 via PE matmul with an
     augmented contract row; R stays in PSUM (no drain).
  2. top-20 per row, chunked: per 512-col chunk DVE max8 + max_index read
     the PSUM tile directly (2 passes over the row total instead of 8).
     The 64 chunk candidates are packed (value mantissa | column index) so
     a 3-round max8/match_replace on the 64-wide union yields the top-20
     indices with no further full-row scans.
  3. EdgeConv1 is linear before the LReLU, so it is precomputed per point:
       conv1(i,j) = Wn x_j + (Wc - Wn) x_i  with BN1 folded in
     A' = s1*(Wn x)        -> fp16 DRAM table, 20 rows gathered per block
                              in ONE indirect DMA
     B' = s1*((Wc-Wn) x)+t1-> fp16 on-chip, broadcast-added per query block
  4. e1 = lrelu(A'_j + B'_i): add on DVE (fp16 2x), lrelu on GPSIMD;
     PE-transpose to channel-major (fp16); EdgeConv2 as fp16 64x64 matmuls
     (BN2 scale folded into W2, bias t2 added during the PSUM drain);
     max over k as fp16 DVE tree; lrelu (monotone, s2 >= 0).
  5. point MLP 64->256->128->1, fp16 weights, BN scales folded, biases
     added during PSUM drains, lrelu on GPSIMD.
"""

import numpy as np

C = 64
K = 20
NEG = 0.2
EPS = 1e-5
NCORES = 8
N_FULL = 4096
NEG_FILL = -3.0e38
NCH = 4          # top-k chunks per 128-row block
CHW = 1024       # chunk width

_cache = {}


def build_nc(n):
    from contextlib import ExitStack

    import concourse.bass as bass
    import concourse.bacc as bacc
    import concourse.mybir as mybir
    import concourse.tile as tile
    from concourse.masks import make_identity

    f32 = mybir.dt.float32
    f32r = mybir.dt.float32r
    f16 = mybir.dt.float16
    u32 = mybir.dt.uint32
    AF = mybir.ActivationFunctionType
    OP = mybir.AluOpType

    nblk = n // 128
    nchk = n // 512

    nc = bacc.Bacc("TRN2", target_bir_lowering=False, debug=False,
                   num_devices=NCORES)

    x_d = nc.dram_tensor("x", [C, n], f32, kind="ExternalInput")
    wnT_d = nc.dram_tensor("wnT", [C, C], f32, kind="ExternalInput")
    wcnT_d = nc.dram_tensor("wcnT", [C, C], f32, kind="ExternalInput")
    t1_d = nc.dram_tensor("t1", [C, 1], f32, kind="ExternalInput")
    w2T_d = nc.dram_tensor("w2T", [C, C], f16, kind="ExternalInput")
    t2_d = nc.dram_tensor("t2", [C, 1], f32, kind="ExternalInput")
    w1aT_d = nc.dram_tensor("w1aT", [C, 128], f16, kind="ExternalInput")
    w1bT_d = nc.dram_tensor("w1bT", [C, 128], f16, kind="ExternalInput")
    tm1a_d = nc.dram_tensor("tm1a", [128, 1], f32, kind="ExternalInput")
    tm1b_d = nc.dram_tensor("tm1b", [128, 1], f32, kind="ExternalInput")
    w2maT_d = nc.dram_tensor("w2maT", [128, 128], f16, kind="ExternalInput")
    w2mbT_d = nc.dram_tensor("w2mbT", [128, 128], f16, kind="ExternalInput")
    tm2_d = nc.dram_tensor("tm2", [128, 1], f32, kind="ExternalInput")
    w3T_d = nc.dram_tensor("w3T", [128, 1], f16, kind="ExternalInput")
    b3_d = nc.dram_tensor("b3", [1, 1], f32, kind="ExternalInput")
    choff_d = nc.dram_tensor("choff", [128, NCH * 8], u32, kind="ExternalInput")
    out_d = nc.dram_tensor("out", [1, n], f32, kind="ExternalOutput")

    with tile.TileContext(nc) as tc, ExitStack() as top:
        cpool = top.enter_context(tc.tile_pool(name="consts", bufs=1))
        dpool = top.enter_context(tc.tile_pool(name="dram", bufs=1, space="DRAM"))
        xpool = top.enter_context(tc.tile_pool(name="xaug", bufs=1))
        hpool = top.enter_context(tc.tile_pool(name="hout", bufs=1))

        # --- constants / weights ---
        ident = cpool.tile([128, 128], f32, tag="ident")
        make_identity(nc, ident[:])
        ident16 = cpool.tile([128, 128], f16, tag="ident16")
        nc.scalar.copy(out=ident16[:], in_=ident[:])
        ones64 = cpool.tile([C, 1], f32, tag="ones64")
        nc.vector.memset(ones64[:], 1.0)

        def load_const(dram, shape, tag, dt=f32):
            t = cpool.tile(shape, dt, tag=tag)
            nc.sync.dma_start(t[:], dram[:])
            return t

        wnT = load_const(wnT_d, [C, C], "wnT")
        wcnT = load_const(wcnT_d, [C, C], "wcnT")
        t1 = load_const(t1_d, [C, 1], "t1")
        w2T = load_const(w2T_d, [C, C], "w2T", f16)
        t2 = load_const(t2_d, [C, 1], "t2")
        w1aT = load_const(w1aT_d, [C, 128], "w1aT", f16)
        w1bT = load_const(w1bT_d, [C, 128], "w1bT", f16)
        tm1a = load_const(tm1a_d, [128, 1], "tm1a")
        tm1b = load_const(tm1b_d, [128, 1], "tm1b")
        w2maT = load_const(w2maT_d, [128, 128], "w2maT", f16)
        w2mbT = load_const(w2mbT_d, [128, 128], "w2mbT", f16)
        tm2 = load_const(tm2_d, [128, 1], "tm2")
        w3T = load_const(w3T_d, [128, 1], "w3T", f16)
        b3 = load_const(b3_d, [1, 1], "b3")
        choff = load_const(choff_d, [128, NCH * 8], "choff", u32)

        At = dpool.tile([n, C], f16, tag="At")           # A' fp16 gather table
        xaug = xpool.tile([C + 1, n], f32, tag="xaug")   # rows 0..63 = x, row 64 = ||x_j||^2
        x2aug = xpool.tile([C + 1, n], f32, tag="x2aug") # rows 0..63 = 2x, row 64 = -1
        Bt = xpool.tile([128, C * nblk], f16, tag="Bt")  # B' fp16, block j at cols 64j
        H = hpool.tile([C, n], f16, tag="H")             # per-point features after edge max
        osb = hpool.tile([1, n], f32, tag="osb")

        # ---------------- stage 0: tables ----------------
        with tc.tile_pool(name="s0sb", bufs=2) as s0sb, \
             tc.tile_pool(name="s0ps", bufs=3, space="PSUM") as s0ps, \
             tc.tile_pool(name="s0pt", bufs=2, space="PSUM") as s0pt:
            # x load split in quarters across queues so chunk 0 lands early
            nq = n // 4
            nc.sync.dma_start(xaug[:C, 0 * nq:1 * nq], x_d[:, 0 * nq:1 * nq])
            nc.scalar.dma_start(xaug[:C, 1 * nq:2 * nq], x_d[:, 1 * nq:2 * nq])
            nc.gpsimd.dma_start(xaug[:C, 2 * nq:3 * nq], x_d[:, 2 * nq:3 * nq])
            nc.sync.dma_start(xaug[:C, 3 * nq:4 * nq], x_d[:, 3 * nq:4 * nq])
            nc.gpsimd.memset(x2aug[C:C + 1, :], -1.0)
            for ch in range(nchk):
                cs = slice(512 * ch, 512 * (ch + 1))
                # 2x copy per chunk (Act), squares on DVE, drains split
                nc.scalar.activation(out=x2aug[:C, cs], in_=xaug[:C, cs],
                                     func=AF.Copy, scale=2.0)
                xsq = s0sb.tile([C, 512], f32, tag="xsq")
                nc.vector.tensor_mul(out=xsq[:], in0=xaug[:C, cs],
                                     in1=xaug[:C, cs])
                psxx = s0ps.tile([1, 512], f32, tag="s0p", space="PSUM")
                nc.tensor.matmul(out=psxx[:], lhsT=ones64[:].bitcast(f32r),
                                 rhs=xsq[:].bitcast(f32r),
                                 start=True, stop=True)
                nc.gpsimd.tensor_copy(out=xaug[C:C + 1, cs], in_=psxx[:])
                psa = s0ps.tile([C, 512], f32, tag="s0p", space="PSUM")
                nc.tensor.matmul(out=psa[:], lhsT=wnT[:].bitcast(f32r),
                                 rhs=xaug[:C, cs].bitcast(f32r),
                                 start=True, stop=True)
                ap = s0sb.tile([C, 512], f16, tag="ap")
                nc.gpsimd.tensor_copy(out=ap[:], in_=psa[:])
                psb = s0ps.tile([C, 512], f32, tag="s0p", space="PSUM")
                nc.tensor.matmul(out=psb[:], lhsT=wcnT[:].bitcast(f32r),
                                 rhs=xaug[:C, cs].bitcast(f32r),
                                 start=True, stop=True)
                bp = s0sb.tile([C, 512], f16, tag="bp")
                nc.vector.tensor_scalar(out=bp[:], in0=psb[:], scalar1=t1[:],
                                        scalar2=None, op0=OP.add)
                ast = s0sb.tile([128, 4 * C], f16, tag="ast")
                for j in range(4):
                    blk = 4 * ch + j
                    js = slice(128 * j, 128 * (j + 1))
                    pta = s0pt.tile([128, C], f16, tag="s0t", space="PSUM")
                    nc.tensor.transpose(out=pta[:], in_=ap[:, js],
                                        identity=ident16[:C, :C])
                    nc.gpsimd.tensor_copy(out=ast[:, C * j:C * (j + 1)],
                                          in_=pta[:])
                    ptb = s0pt.tile([128, C], f16, tag="s0t", space="PSUM")
                    nc.tensor.transpose(out=ptb[:], in_=bp[:, js],
                                        identity=ident16[:C, :C])
                    nc.scalar.copy(out=Bt[:, C * blk:C * (blk + 1)], in_=ptb[:])
                nc.sync.dma_start(
                    At[512 * ch:512 * (ch + 1), :].rearrange(
                        "(j p) c -> p j c", p=128),
                    ast[:].rearrange("p (j c) -> p j c", j=4))

        # ---------------- stage 1: blocks ----------------
        with tc.tile_pool(name="cpoolv", bufs=2) as cvp, \
             tc.tile_pool(name="gpool", bufs=2) as gpool, \
             tc.tile_pool(name="epool", bufs=2) as epool, \
             tc.tile_pool(name="wpool", bufs=2) as wpool, \
             tc.tile_pool(name="tpool", bufs=2) as tpool, \
             tc.tile_pool(name="mlpsb", bufs=2) as mlpsb, \
             tc.tile_pool(name="psR", bufs=2, space="PSUM") as psR, \
             tc.tile_pool(name="psT", bufs=2, space="PSUM") as psT, \
             tc.tile_pool(name="psE", bufs=1, space="PSUM") as psE, \
             tc.tile_pool(name="mlpps", bufs=1, space="PSUM") as mlpps:

            def mlp_chunk(ch):
                cs = slice(512 * ch, 512 * (ch + 1))
                l1a = mlpsb.tile([128, 512], f16, tag="l1a")
                l1b = mlpsb.tile([128, 512], f16, tag="l1b")
                ps1a = mlpps.tile([128, 512], f32, tag="mlpp", space="PSUM")
                nc.tensor.matmul(out=ps1a[:], lhsT=w1aT[:], rhs=H[:, cs],
                                 start=True, stop=True)
                nc.scalar.activation(out=l1a[:], in_=ps1a[:],
                                     func=AF.Identity, bias=tm1a[:], scale=1.0)
                nc.gpsimd.scalar_tensor_tensor(
                    out=l1a[:], in0=l1a[:], scalar=NEG, in1=l1a[:],
                    op0=OP.mult, op1=OP.max)
                ps1b = mlpps.tile([128, 512], f32, tag="mlpp", space="PSUM")
                nc.tensor.matmul(out=ps1b[:], lhsT=w1bT[:], rhs=H[:, cs],
                                 start=True, stop=True)
                nc.scalar.activation(out=l1b[:], in_=ps1b[:],
                                     func=AF.Identity, bias=tm1b[:], scale=1.0)
                nc.gpsimd.scalar_tensor_tensor(
                    out=l1b[:], in0=l1b[:], scalar=NEG, in1=l1b[:],
                    op0=OP.mult, op1=OP.max)
                ps2 = mlpps.tile([128, 512], f32, tag="mlpp", space="PSUM")
                nc.tensor.matmul(out=ps2[:], lhsT=w2maT[:], rhs=l1a[:],
                                 start=True, stop=False)
                nc.tensor.matmul(out=ps2[:], lhsT=w2mbT[:], rhs=l1b[:],
                                 start=False, stop=True)
                l2 = mlpsb.tile([128, 512], f16, tag="l2")
                nc.scalar.activation(out=l2[:], in_=ps2[:],
                                     func=AF.Identity, bias=tm2[:], scale=1.0)
                nc.gpsimd.scalar_tensor_tensor(
                    out=l2[:], in0=l2[:], scalar=NEG, in1=l2[:],
                    op0=OP.mult, op1=OP.max)
                ps3 = mlpps.tile([1, 512], f32, tag="mlpp", space="PSUM")
                nc.tensor.matmul(out=ps3[:], lhsT=w3T[:], rhs=l2[:],
                                 start=True, stop=True)
                nc.scalar.activation(out=osb[:, cs], in_=ps3[:],
                                     func=AF.Identity, bias=b3[:], scale=1.0)

            for b in range(nblk):
                bs = slice(128 * b, 128 * (b + 1))

                # pairwise + chunked top-8, straight from PSUM
                cand_v = cvp.tile([128, NCH * 8], f32, tag="cv")
                cand_i = cvp.tile([128, NCH * 8], u32, tag="ci")
                for ch in range(NCH):
                    ks = slice(8 * ch, 8 * (ch + 1))
                    ps = psR.tile([128, CHW], f32, tag="psr", space="PSUM")
                    for hh in range(CHW // 512):
                        c0 = CHW * ch + 512 * hh
                        nc.tensor.matmul(
                            out=ps[:, 512 * hh:512 * (hh + 1)],
                            lhsT=x2aug[:, bs].bitcast(f32r),
                            rhs=xaug[:, c0:c0 + 512].bitcast(f32r),
                            start=True, stop=True)
                    nc.vector.max(out=cand_v[:, ks], in_=ps[:])
                    nc.vector.max_index(out=cand_i[:, ks],
                                        in_max=cand_v[:, ks], in_values=ps[:])

                # pack value|index on GPSIMD, union top-20 on DVE
                pk = cvp.tile([128, NCH * 8], u32, tag="pk")
                nc.gpsimd.tensor_tensor(out=cand_i[:], in0=cand_i[:],
                                        in1=choff[:], op=OP.add)
                nc.gpsimd.tensor_scalar(out=pk[:], in0=cand_v[:].bitcast(u32),
                                        scalar1=0xFFFFF000, scalar2=None,
                                        op0=OP.bitwise_and)
                nc.gpsimd.tensor_tensor(out=pk[:], in0=pk[:], in1=cand_i[:],
                                        op=OP.bitwise_or)
                pkf = pk[:].bitcast(f32)
                pv1 = cvp.tile([128, 8], f32, tag="pv1")
                pv2 = cvp.tile([128, 8], f32, tag="pv2")
                pv3 = cvp.tile([128, 8], f32, tag="pv3")
                nc.vector.max(out=pv1[:], in_=pkf)
                nc.vector.match_replace(out=pkf, in_to_replace=pv1[:],
                                        in_values=pkf, imm_value=NEG_FILL)
                nc.vector.max(out=pv2[:], in_=pkf)
                nc.vector.match_replace(out=pkf, in_to_replace=pv2[:],
                                        in_values=pkf, imm_value=NEG_FILL)
                nc.vector.max(out=pv3[:], in_=pkf)
                idx = cvp.tile([128, 24], u32, tag="idx")
                nc.gpsimd.tensor_scalar(out=idx[:, 0:8], in0=pv1[:].bitcast(u32),
                                        scalar1=0xFFF, scalar2=None,
                                        op0=OP.bitwise_and)
                nc.gpsimd.tensor_scalar(out=idx[:, 8:16], in0=pv2[:].bitcast(u32),
                                        scalar1=0xFFF, scalar2=None,
                                        op0=OP.bitwise_and)
                nc.gpsimd.tensor_scalar(out=idx[:, 16:24], in0=pv3[:].bitcast(u32),
                                        scalar1=0xFFF, scalar2=None,
                                        op0=OP.bitwise_and)

                # gather all 20 neighbors in one indirect DMA (fp16 rows)
                G = gpool.tile([128, K * C], f16, tag="G")
                nc.gpsimd.indirect_dma_start(
                    out=G[:].rearrange("p (k c) -> p k c", k=K),
                    out_offset=None,
                    in_=At[:],
                    in_offset=bass.IndirectOffsetOnAxis(ap=idx[:, 0:K], axis=0))

                # e1 = lrelu(G + B'_i): add + lrelu both on GPSIMD
                bb = Bt[:, C * b:C * (b + 1)].rearrange(
                    "p (k c) -> p k c", k=1).to_broadcast([128, K, C])
                nc.gpsimd.tensor_tensor(
                    out=G[:].rearrange("p (k c) -> p k c", k=K),
                    in0=G[:].rearrange("p (k c) -> p k c", k=K),
                    in1=bb, op=OP.add)
                nc.gpsimd.scalar_tensor_tensor(
                    out=G[:], in0=G[:], scalar=NEG, in1=G[:],
                    op0=OP.mult, op1=OP.max)

                # transpose to channel-major: 20 PE transposes [128,64]->[64,128]
                e1T = epool.tile([C, K * 128], f16, tag="e1T")
                for grp, gw in ((0, 8), (1, 8), (2, 4)):
                    pt = psT.tile([C, 1024], f16, tag="pst", space="PSUM")
                    for s in range(gw):
                        k = 8 * grp + s
                        nc.tensor.transpose(
                            out=pt[:, 128 * s:128 * (s + 1)],
                            in_=G[:, C * k:C * (k + 1)],
                            identity=ident16[:])
                    nc.scalar.copy(
                        out=e1T[:, 1024 * grp:1024 * grp + 128 * gw],
                        in_=pt[:, :128 * gw])

                # conv2 (w_k2 with bn2 scale folded), t2 added in drain
                ew = wpool.tile([C, K * 128], f16, tag="ew")
                for grp in range(5):
                    pe = psE.tile([C, 512], f32, tag="pse", space="PSUM")
                    for s in range(4):
                        k = 4 * grp + s
                        nc.tensor.matmul(
                            out=pe[:, 128 * s:128 * (s + 1)],
                            lhsT=w2T[:],
                            rhs=e1T[:, 128 * k:128 * (k + 1)],
                            start=True, stop=True)
                    nc.scalar.activation(
                        out=ew[:, 512 * grp:512 * (grp + 1)], in_=pe[:],
                        func=AF.Identity, bias=t2[:], scale=1.0)

                # max over k: levels 1-2 on GPSIMD, rest + lrelu on DVE
                m1 = tpool.tile([C, 10 * 128], f16, tag="m1")
                nc.gpsimd.tensor_tensor(out=m1[:], in0=ew[:, :1280],
                                        in1=ew[:, 1280:], op=OP.max)
                m2 = tpool.tile([C, 5 * 128], f16, tag="m2")
                nc.gpsimd.tensor_tensor(out=m2[:], in0=m1[:, :640],
                                        in1=m1[:, 640:], op=OP.max)
                m3 = tpool.tile([C, 2 * 128], f16, tag="m3")
                nc.gpsimd.tensor_tensor(out=m3[:], in0=m2[:, :256],
                                        in1=m2[:, 256:512], op=OP.max)
                m4 = tpool.tile([C, 128], f16, tag="m4")
                nc.gpsimd.tensor_tensor(out=m4[:], in0=m3[:, :128],
                                        in1=m3[:, 128:], op=OP.max)
                nc.gpsimd.tensor_tensor(out=m4[:], in0=m4[:],
                                        in1=m2[:, 512:], op=OP.max)
                nc.gpsimd.scalar_tensor_tensor(
                    out=H[:, bs], in0=m4[:], scalar=NEG, in1=m4[:],
                    op0=OP.mult, op1=OP.max)

                # point MLP for the finished 512-col chunk, interleaved
                if b % 4 == 3:
                    mlp_chunk(b // 4)

            nc.sync.dma_start(out_d[:], osb[:])

    nc.finalize()
    return nc


def host_weights(w_k1, g_k1, b_k1, m_k1, v_k1, w_k2, g_k2, b_k2, m_k2, v_k2,
                 w1, g1, b1, m1, v1, w2, g2, b2, m2, v2, w3, b3):
    f = np.float32
    h = np.float16
    s1 = (g_k1 / np.sqrt(v_k1 + f(EPS))).astype(f)
    t1 = (b_k1 - m_k1 * s1).astype(f)
    wn = w_k1[:, :C]
    wc = w_k1[:, C:]
    wnT = np.ascontiguousarray((wn * s1[:, None]).T.astype(f))
    wcnT = np.ascontiguousarray(((wc - wn) * s1[:, None]).T.astype(f))
    s2 = (g_k2 / np.sqrt(v_k2 + f(EPS))).astype(f)
    t2 = (b_k2 - m_k2 * s2).astype(f)
    w2T = np.ascontiguousarray((w_k2 * s2[:, None]).T.astype(h))
    sm1 = (g1 / np.sqrt(v1 + f(EPS))).astype(f)
    tm1 = (b1 - m1 * sm1).astype(f)
    w1s = (w1 * sm1[:, None]).astype(f)            # (256, 64)
    w1aT = np.ascontiguousarray(w1s[:128].T.astype(h))  # (64, 128)
    w1bT = np.ascontiguousarray(w1s[128:].T.astype(h))
    sm2 = (g2 / np.sqrt(v2 + f(EPS))).astype(f)
    tm2 = (b2 - m2 * sm2).astype(f)
    w2s = (w2 * sm2[:, None]).astype(f)            # (128, 256)
    w2maT = np.ascontiguousarray(w2s[:, :128].T.astype(h))  # (128, 128)
    w2mbT = np.ascontiguousarray(w2s[:, 128:].T.astype(h))
    w3T = np.ascontiguousarray(w3.T.astype(h))     # (128, 1)
    choff = np.broadcast_to(
        (np.repeat(np.arange(NCH, dtype=np.uint32), 8) * CHW)[None, :],
        (128, NCH * 8))
    return {
        "wnT": wnT, "wcnT": wcnT, "t1": t1.reshape(C, 1),
        "w2T": w2T, "t2": t2.reshape(C, 1),
        "w1aT": w1aT, "w1bT": w1bT,
        "tm1a": tm1[:128].reshape(128, 1), "tm1b": tm1[128:].reshape(128, 1),
        "w2maT": w2maT, "w2mbT": w2mbT, "tm2": tm2.reshape(128, 1),
        "w3T": w3T, "b3": b3.reshape(1, 1).astype(f),
        "choff": np.ascontiguousarray(choff),
    }


def kernel(**inputs):
    from concourse.bass_utils import run_bass_kernel_spmd

    x = np.asarray(inputs["x"], dtype=np.float32)  # (B, C, N)
    B = x.shape[0]
    n = x.shape[2]
    w = host_weights(**{k: np.asarray(v, dtype=np.float32)
                        for k, v in inputs.items() if k != "x"})
    if n not in _cache:
        _cache[n] = build_nc(n)
    nc = _cache[n]
    in_maps = [{"x": np.ascontiguousarray(x[c]), **w} for c in range(B)]
    res = run_bass_kernel_spmd(nc, in_maps, list(range(NCORES)))
    out = np.stack([res.results[c]["out"][0] for c in range(B)], axis=0)
    return out.astype(np.float32)


# revision 27
# speedup vs baseline: 3.8179x; 1.0101x over previous
"""DGCNN prediction head on 8 Trainium2 NeuronCores.

Data-parallel over batch B=8: each core runs the full pipeline for one
sample (C=64 channels, N=4096 points, k=20 neighbors).

Per-core pipeline (all on one NeuronCore, no collectives):
  1. pairwise ranking R[i,j] = 2<x_i,x_j> - ||x_j||^2 via PE matmul with an
     augmented contract row; R stays in PSUM (no drain).
  2. top-20 per row, chunked: per 512-col chunk DVE max8 + max_index read
     the PSUM tile directly (2 passes over the row total instead of 8).
     The 64 chunk candidates are packed (value mantissa | column index) so
     a 3-round max8/match_replace on the 64-wide union yields the top-20
     indices with no further full-row scans.
  3. EdgeConv1 is linear before the LReLU, so it is precomputed per point:
       conv1(i,j) = Wn x_j + (Wc - Wn) x_i  with BN1 folded in
     A' = s1*(Wn x)        -> fp16 DRAM table, 20 rows gathered per block
                              in ONE indirect DMA
     B' = s1*((Wc-Wn) x)+t1-> fp16 on-chip, broadcast-added per query block
  4. e1 = lrelu(A'_j + B'_i): add on DVE (fp16 2x), lrelu on GPSIMD;
     PE-transpose to channel-major (fp16); EdgeConv2 as fp16 64x64 matmuls
     (BN2 scale folded into W2, bias t2 added during the PSUM drain);
     max over k as fp16 DVE tree; lrelu (monotone, s2 >= 0).
  5. point MLP 64->256->128->1, fp16 weights, BN scales folded, biases
     added during PSUM drains, lrelu on GPSIMD.
"""

import numpy as np

C = 64
K = 20
NEG = 0.2
EPS = 1e-5
NCORES = 8
N_FULL = 4096
NEG_FILL = -3.0e38
NCH = 4          # top-k chunks per 128-row block
CHW = 1024       # chunk width

_cache = {}


def build_nc(n):
    from contextlib import ExitStack

    import concourse.bass as bass
    import concourse.bacc as bacc
    import concourse.mybir as mybir
    import concourse.tile as tile
    from concourse.masks import make_identity

    f32 = mybir.dt.float32
    f32r = mybir.dt.float32r
    f16 = mybir.dt.float16
    u32 = mybir.dt.uint32
    AF = mybir.ActivationFunctionType
    OP = mybir.AluOpType

    nblk = n // 128
    nchk = n // 512

    nc = bacc.Bacc("TRN2", target_bir_lowering=False, debug=False,
                   num_devices=NCORES)

    x_d = nc.dram_tensor("x", [C, n], f32, kind="ExternalInput")
    wnT_d = nc.dram_tensor("wnT", [C, C], f32, kind="ExternalInput")
    wcnT_d = nc.dram_tensor("wcnT", [C, C], f32, kind="ExternalInput")
    t1_d = nc.dram_tensor("t1", [C, 1], f32, kind="ExternalInput")
    w2T_d = nc.dram_tensor("w2T", [C, C], f16, kind="ExternalInput")
    t2_d = nc.dram_tensor("t2", [C, 1], f32, kind="ExternalInput")
    w1aT_d = nc.dram_tensor("w1aT", [C, 128], f16, kind="ExternalInput")
    w1bT_d = nc.dram_tensor("w1bT", [C, 128], f16, kind="ExternalInput")
    tm1a_d = nc.dram_tensor("tm1a", [128, 1], f32, kind="ExternalInput")
    tm1b_d = nc.dram_tensor("tm1b", [128, 1], f32, kind="ExternalInput")
    w2maT_d = nc.dram_tensor("w2maT", [128, 128], f16, kind="ExternalInput")
    w2mbT_d = nc.dram_tensor("w2mbT", [128, 128], f16, kind="ExternalInput")
    tm2_d = nc.dram_tensor("tm2", [128, 1], f32, kind="ExternalInput")
    w3T_d = nc.dram_tensor("w3T", [128, 1], f16, kind="ExternalInput")
    b3_d = nc.dram_tensor("b3", [1, 1], f32, kind="ExternalInput")
    choff_d = nc.dram_tensor("choff", [128, NCH * 8], u32, kind="ExternalInput")
    out_d = nc.dram_tensor("out", [1, n], f32, kind="ExternalOutput")

    with tile.TileContext(nc) as tc, ExitStack() as top:
        cpool = top.enter_context(tc.tile_pool(name="consts", bufs=1))
        dpool = top.enter_context(tc.tile_pool(name="dram", bufs=1, space="DRAM"))
        xpool = top.enter_context(tc.tile_pool(name="xaug", bufs=1))
        hpool = top.enter_context(tc.tile_pool(name="hout", bufs=1))

        # --- x load issued before const loads so SP serves stage-0 ASAP;
        # quarters across queues so chunk 0 lands early ---
        nq = n // 4
        xaug = xpool.tile([C + 1, n], f32, tag="xaug")   # rows 0..63 = x, row 64 = ||x_j||^2
        nc.sync.dma_start(xaug[:C, 0 * nq:1 * nq], x_d[:, 0 * nq:1 * nq])
        nc.scalar.dma_start(xaug[:C, 1 * nq:2 * nq], x_d[:, 1 * nq:2 * nq])
        nc.gpsimd.dma_start(xaug[:C, 2 * nq:3 * nq], x_d[:, 2 * nq:3 * nq])
        nc.sync.dma_start(xaug[:C, 3 * nq:4 * nq], x_d[:, 3 * nq:4 * nq])

        # --- constants / weights ---
        ident = cpool.tile([128, 128], f32, tag="ident")
        make_identity(nc, ident[:])
        ident16 = cpool.tile([128, 128], f16, tag="ident16")
        nc.scalar.copy(out=ident16[:], in_=ident[:])
        ones64 = cpool.tile([C, 1], f32, tag="ones64")
        nc.vector.memset(ones64[:], 1.0)

        def load_const(dram, shape, tag, dt=f32):
            t = cpool.tile(shape, dt, tag=tag)
            nc.sync.dma_start(t[:], dram[:])
            return t

        wnT = load_const(wnT_d, [C, C], "wnT")
        wcnT = load_const(wcnT_d, [C, C], "wcnT")
        t1 = load_const(t1_d, [C, 1], "t1")
        w2T = load_const(w2T_d, [C, C], "w2T", f16)
        t2 = load_const(t2_d, [C, 1], "t2")
        w1aT = load_const(w1aT_d, [C, 128], "w1aT", f16)
        w1bT = load_const(w1bT_d, [C, 128], "w1bT", f16)
        tm1a = load_const(tm1a_d, [128, 1], "tm1a")
        tm1b = load_const(tm1b_d, [128, 1], "tm1b")
        w2maT = load_const(w2maT_d, [128, 128], "w2maT", f16)
        w2mbT = load_const(w2mbT_d, [128, 128], "w2mbT", f16)
        tm2 = load_const(tm2_d, [128, 1], "tm2")
        w3T = load_const(w3T_d, [128, 1], "w3T", f16)
        b3 = load_const(b3_d, [1, 1], "b3")
        choff = load_const(choff_d, [128, NCH * 8], "choff", u32)

        At = dpool.tile([n, C], f16, tag="At")           # A' fp16 gather table
        x2aug = xpool.tile([C + 1, n], f32, tag="x2aug") # rows 0..63 = 2x, row 64 = -1
        Bt = xpool.tile([128, C * nblk], f16, tag="Bt")  # B' fp16, block j at cols 64j
        H = hpool.tile([C, n], f16, tag="H")             # per-point features after edge max
        osb = hpool.tile([1, n], f32, tag="osb")

        # ---------------- stage 0: tables ----------------
        with tc.tile_pool(name="s0sb", bufs=2) as s0sb, \
             tc.tile_pool(name="s0ps", bufs=3, space="PSUM") as s0ps, \
             tc.tile_pool(name="s0pt", bufs=2, space="PSUM") as s0pt:
            nc.gpsimd.memset(x2aug[C:C + 1, :], -1.0)
            for ch in range(nchk):
                cs = slice(512 * ch, 512 * (ch + 1))
                # 2x copy + squares per chunk (Act), drains split
                nc.scalar.activation(out=x2aug[:C, cs], in_=xaug[:C, cs],
                                     func=AF.Copy, scale=2.0)
                xsq = s0sb.tile([C, 512], f32, tag="xsq")
                nc.scalar.activation(out=xsq[:], in_=xaug[:C, cs],
                                     func=AF.Square)
                psxx = s0ps.tile([1, 512], f32, tag="s0p", space="PSUM")
                nc.tensor.matmul(out=psxx[:], lhsT=ones64[:].bitcast(f32r),
                                 rhs=xsq[:].bitcast(f32r),
                                 start=True, stop=True)
                nc.gpsimd.tensor_copy(out=xaug[C:C + 1, cs], in_=psxx[:])
                psa = s0ps.tile([C, 512], f32, tag="s0p", space="PSUM")
                nc.tensor.matmul(out=psa[:], lhsT=wnT[:].bitcast(f32r),
                                 rhs=xaug[:C, cs].bitcast(f32r),
                                 start=True, stop=True)
                ap = s0sb.tile([C, 512], f16, tag="ap")
                nc.gpsimd.tensor_copy(out=ap[:], in_=psa[:])
                psb = s0ps.tile([C, 512], f32, tag="s0p", space="PSUM")
                nc.tensor.matmul(out=psb[:], lhsT=wcnT[:].bitcast(f32r),
                                 rhs=xaug[:C, cs].bitcast(f32r),
                                 start=True, stop=True)
                bp = s0sb.tile([C, 512], f16, tag="bp")
                nc.vector.tensor_scalar(out=bp[:], in0=psb[:], scalar1=t1[:],
                                        scalar2=None, op0=OP.add)
                ast = s0sb.tile([128, 4 * C], f16, tag="ast")
                for j in range(4):
                    blk = 4 * ch + j
                    js = slice(128 * j, 128 * (j + 1))
                    pta = s0pt.tile([128, C], f16, tag="s0t", space="PSUM")
                    nc.tensor.transpose(out=pta[:], in_=ap[:, js],
                                        identity=ident16[:C, :C])
                    nc.gpsimd.tensor_copy(out=ast[:, C * j:C * (j + 1)],
                                          in_=pta[:])
                    ptb = s0pt.tile([128, C], f16, tag="s0t", space="PSUM")
                    nc.tensor.transpose(out=ptb[:], in_=bp[:, js],
                                        identity=ident16[:C, :C])
                    nc.scalar.copy(out=Bt[:, C * blk:C * (blk + 1)], in_=ptb[:])
                nc.sync.dma_start(
                    At[512 * ch:512 * (ch + 1), :].rearrange(
                        "(j p) c -> p j c", p=128),
                    ast[:].rearrange("p (j c) -> p j c", j=4))

        # ---------------- stage 1: blocks ----------------
        with tc.tile_pool(name="cpoolv", bufs=2) as cvp, \
             tc.tile_pool(name="gpool", bufs=2) as gpool, \
             tc.tile_pool(name="epool", bufs=2) as epool, \
             tc.tile_pool(name="wpool", bufs=2) as wpool, \
             tc.tile_pool(name="tpool", bufs=2) as tpool, \
             tc.tile_pool(name="mlpsb", bufs=2) as mlpsb, \
             tc.tile_pool(name="psR", bufs=2, space="PSUM") as psR, \
             tc.tile_pool(name="psT", bufs=2, space="PSUM") as psT, \
             tc.tile_pool(name="psE", bufs=1, space="PSUM") as psE, \
             tc.tile_pool(name="mlpps", bufs=1, space="PSUM") as mlpps:

            def mlp_chunk(ch):
                cs = slice(512 * ch, 512 * (ch + 1))
                l1a = mlpsb.tile([128, 512], f16, tag="l1a")
                l1b = mlpsb.tile([128, 512], f16, tag="l1b")
                ps1a = mlpps.tile([128, 512], f32, tag="mlpp", space="PSUM")
                nc.tensor.matmul(out=ps1a[:], lhsT=w1aT[:], rhs=H[:, cs],
                                 start=True, stop=True)
                nc.scalar.activation(out=l1a[:], in_=ps1a[:],
                                     func=AF.Identity, bias=tm1a[:], scale=1.0)
                nc.gpsimd.scalar_tensor_tensor(
                    out=l1a[:], in0=l1a[:], scalar=NEG, in1=l1a[:],
                    op0=OP.mult, op1=OP.max)
                ps1b = mlpps.tile([128, 512], f32, tag="mlpp", space="PSUM")
                nc.tensor.matmul(out=ps1b[:], lhsT=w1bT[:], rhs=H[:, cs],
                                 start=True, stop=True)
                nc.scalar.activation(out=l1b[:], in_=ps1b[:],
                                     func=AF.Identity, bias=tm1b[:], scale=1.0)
                nc.gpsimd.scalar_tensor_tensor(
                    out=l1b[:], in0=l1b[:], scalar=NEG, in1=l1b[:],
                    op0=OP.mult, op1=OP.max)
                ps2 = mlpps.tile([128, 512], f32, tag="mlpp", space="PSUM")
                nc.tensor.matmul(out=ps2[:], lhsT=w2maT[:], rhs=l1a[:],
                                 start=True, stop=False)
                nc.tensor.matmul(out=ps2[:], lhsT=w2mbT[:], rhs=l1b[:],
                                 start=False, stop=True)
                l2 = mlpsb.tile([128, 512], f16, tag="l2")
                nc.scalar.activation(out=l2[:], in_=ps2[:],
                                     func=AF.Identity, bias=tm2[:], scale=1.0)
                nc.gpsimd.scalar_tensor_tensor(
                    out=l2[:], in0=l2[:], scalar=NEG, in1=l2[:],
                    op0=OP.mult, op1=OP.max)
                ps3 = mlpps.tile([1, 512], f32, tag="mlpp", space="PSUM")
                nc.tensor.matmul(out=ps3[:], lhsT=w3T[:], rhs=l2[:],
                                 start=True, stop=True)
                nc.scalar.activation(out=osb[:, cs], in_=ps3[:],
                                     func=AF.Identity, bias=b3[:], scale=1.0)

            for b in range(nblk):
                bs = slice(128 * b, 128 * (b + 1))

                # pairwise + chunked top-8, straight from PSUM
                cand_v = cvp.tile([128, NCH * 8], f32, tag="cv")
                cand_i = cvp.tile([128, NCH * 8], u32, tag="ci")
                for ch in range(NCH):
                    ks = slice(8 * ch, 8 * (ch + 1))
                    ps = psR.tile([128, CHW], f32, tag="psr", space="PSUM")
                    for hh in range(CHW // 512):
                        c0 = CHW * ch + 512 * hh
                        nc.tensor.matmul(
                            out=ps[:, 512 * hh:512 * (hh + 1)],
                            lhsT=x2aug[:, bs].bitcast(f32r),
                            rhs=xaug[:, c0:c0 + 512].bitcast(f32r),
                            start=True, stop=True)
                    nc.vector.max(out=cand_v[:, ks], in_=ps[:])
                    nc.vector.max_index(out=cand_i[:, ks],
                                        in_max=cand_v[:, ks], in_values=ps[:])

                # pack value|index on GPSIMD, union top-20 on DVE
                pk = cvp.tile([128, NCH * 8], u32, tag="pk")
                nc.gpsimd.tensor_tensor(out=cand_i[:], in0=cand_i[:],
                                        in1=choff[:], op=OP.add)
                nc.gpsimd.tensor_scalar(out=pk[:], in0=cand_v[:].bitcast(u32),
                                        scalar1=0xFFFFF000, scalar2=None,
                                        op0=OP.bitwise_and)
                nc.gpsimd.tensor_tensor(out=pk[:], in0=pk[:], in1=cand_i[:],
                                        op=OP.bitwise_or)
                pkf = pk[:].bitcast(f32)
                pv1 = cvp.tile([128, 8], f32, tag="pv1")
                pv2 = cvp.tile([128, 8], f32, tag="pv2")
                pv3 = cvp.tile([128, 8], f32, tag="pv3")
                nc.vector.max(out=pv1[:], in_=pkf)
                nc.vector.match_replace(out=pkf, in_to_replace=pv1[:],
                                        in_values=pkf, imm_value=NEG_FILL)
                nc.vector.max(out=pv2[:], in_=pkf)
                nc.vector.match_replace(out=pkf, in_to_replace=pv2[:],
                                        in_values=pkf, imm_value=NEG_FILL)
                nc.vector.max(out=pv3[:], in_=pkf)
                idx = cvp.tile([128, 24], u32, tag="idx")
                nc.gpsimd.tensor_scalar(out=idx[:, 0:8], in0=pv1[:].bitcast(u32),
                                        scalar1=0xFFF, scalar2=None,
                                        op0=OP.bitwise_and)
                nc.gpsimd.tensor_scalar(out=idx[:, 8:16], in0=pv2[:].bitcast(u32),
                                        scalar1=0xFFF, scalar2=None,
                                        op0=OP.bitwise_and)
                nc.gpsimd.tensor_scalar(out=idx[:, 16:24], in0=pv3[:].bitcast(u32),
                                        scalar1=0xFFF, scalar2=None,
                                        op0=OP.bitwise_and)

                # gather all 20 neighbors in one indirect DMA (fp16 rows)
                G = gpool.tile([128, K * C], f16, tag="G")
                nc.gpsimd.indirect_dma_start(
                    out=G[:].rearrange("p (k c) -> p k c", k=K),
                    out_offset=None,
                    in_=At[:],
                    in_offset=bass.IndirectOffsetOnAxis(ap=idx[:, 0:K], axis=0))

                # e1 = lrelu(G + B'_i): add + lrelu both on GPSIMD
                bb = Bt[:, C * b:C * (b + 1)].rearrange(
                    "p (k c) -> p k c", k=1).to_broadcast([128, K, C])
                nc.gpsimd.tensor_tensor(
                    out=G[:].rearrange("p (k c) -> p k c", k=K),
                    in0=G[:].rearrange("p (k c) -> p k c", k=K),
                    in1=bb, op=OP.add)
                nc.gpsimd.scalar_tensor_tensor(
                    out=G[:], in0=G[:], scalar=NEG, in1=G[:],
                    op0=OP.mult, op1=OP.max)

                # transpose to channel-major: 20 PE transposes [128,64]->[64,128]
                e1T = epool.tile([C, K * 128], f16, tag="e1T")
                for grp, gw in ((0, 8), (1, 8), (2, 4)):
                    pt = psT.tile([C, 1024], f16, tag="pst", space="PSUM")
                    for s in range(gw):
                        k = 8 * grp + s
                        nc.tensor.transpose(
                            out=pt[:, 128 * s:128 * (s + 1)],
                            in_=G[:, C * k:C * (k + 1)],
                            identity=ident16[:])
                    nc.scalar.copy(
                        out=e1T[:, 1024 * grp:1024 * grp + 128 * gw],
                        in_=pt[:, :128 * gw])

                # conv2 (w_k2 with bn2 scale folded), t2 added in drain
                ew = wpool.tile([C, K * 128], f16, tag="ew")
                for grp in range(5):
                    pe = psE.tile([C, 512], f32, tag="pse", space="PSUM")
                    for s in range(4):
                        k = 4 * grp + s
                        nc.tensor.matmul(
                            out=pe[:, 128 * s:128 * (s + 1)],
                            lhsT=w2T[:],
                            rhs=e1T[:, 128 * k:128 * (k + 1)],
                            start=True, stop=True)
                    nc.scalar.activation(
                        out=ew[:, 512 * grp:512 * (grp + 1)], in_=pe[:],
                        func=AF.Identity, bias=t2[:], scale=1.0)

                # max over k: levels 1-2 on GPSIMD, rest + lrelu on DVE
                m1 = tpool.tile([C, 10 * 128], f16, tag="m1")
                nc.gpsimd.tensor_tensor(out=m1[:], in0=ew[:, :1280],
                                        in1=ew[:, 1280:], op=OP.max)
                m2 = tpool.tile([C, 5 * 128], f16, tag="m2")
                nc.gpsimd.tensor_tensor(out=m2[:], in0=m1[:, :640],
                                        in1=m1[:, 640:], op=OP.max)
                m3 = tpool.tile([C, 2 * 128], f16, tag="m3")
                nc.gpsimd.tensor_tensor(out=m3[:], in0=m2[:, :256],
                                        in1=m2[:, 256:512], op=OP.max)
                m4 = tpool.tile([C, 128], f16, tag="m4")
                nc.gpsimd.tensor_tensor(out=m4[:], in0=m3[:, :128],
                                        in1=m3[:, 128:], op=OP.max)
                nc.gpsimd.tensor_tensor(out=m4[:], in0=m4[:],
                                        in1=m2[:, 512:], op=OP.max)
                nc.gpsimd.scalar_tensor_tensor(
                    out=H[:, bs], in0=m4[:], scalar=NEG, in1=m4[:],
                    op0=OP.mult, op1=OP.max)

                # point MLP for the finished 512-col chunk, interleaved
                if b % 4 == 3:
                    mlp_chunk(b // 4)

            nc.sync.dma_start(out_d[:], osb[:])

    nc.finalize()
    return nc


def host_weights(w_k1, g_k1, b_k1, m_k1, v_k1, w_k2, g_k2, b_k2, m_k2, v_k2,
                 w1, g1, b1, m1, v1, w2, g2, b2, m2, v2, w3, b3):
    f = np.float32
    h = np.float16
    s1 = (g_k1 / np.sqrt(v_k1 + f(EPS))).astype(f)
    t1 = (b_k1 - m_k1 * s1).astype(f)
    wn = w_k1[:, :C]
    wc = w_k1[:, C:]
    wnT = np.ascontiguousarray((wn * s1[:, None]).T.astype(f))
    wcnT = np.ascontiguousarray(((wc - wn) * s1[:, None]).T.astype(f))
    s2 = (g_k2 / np.sqrt(v_k2 + f(EPS))).astype(f)
    t2 = (b_k2 - m_k2 * s2).astype(f)
    w2T = np.ascontiguousarray((w_k2 * s2[:, None]).T.astype(h))
    sm1 = (g1 / np.sqrt(v1 + f(EPS))).astype(f)
    tm1 = (b1 - m1 * sm1).astype(f)
    w1s = (w1 * sm1[:, None]).astype(f)            # (256, 64)
    w1aT = np.ascontiguousarray(w1s[:128].T.astype(h))  # (64, 128)
    w1bT = np.ascontiguousarray(w1s[128:].T.astype(h))
    sm2 = (g2 / np.sqrt(v2 + f(EPS))).astype(f)
    tm2 = (b2 - m2 * sm2).astype(f)
    w2s = (w2 * sm2[:, None]).astype(f)            # (128, 256)
    w2maT = np.ascontiguousarray(w2s[:, :128].T.astype(h))  # (128, 128)
    w2mbT = np.ascontiguousarray(w2s[:, 128:].T.astype(h))
    w3T = np.ascontiguousarray(w3.T.astype(h))     # (128, 1)
    choff = np.broadcast_to(
        (np.repeat(np.arange(NCH, dtype=np.uint32), 8) * CHW)[None, :],
        (128, NCH * 8))
    return {
        "wnT": wnT, "wcnT": wcnT, "t1": t1.reshape(C, 1),
        "w2T": w2T, "t2": t2.reshape(C, 1),
        "w1aT": w1aT, "w1bT": w1bT,
        "tm1a": tm1[:128].reshape(128, 1), "tm1b": tm1[128:].reshape(128, 1),
        "w2maT": w2maT, "w2mbT": w2mbT, "tm2": tm2.reshape(128, 1),
        "w3T": w3T, "b3": b3.reshape(1, 1).astype(f),
        "choff": np.ascontiguousarray(choff),
    }


def kernel(**inputs):
    from concourse.bass_utils import run_bass_kernel_spmd

    x = np.asarray(inputs["x"], dtype=np.float32)  # (B, C, N)
    B = x.shape[0]
    n = x.shape[2]
    w = host_weights(**{k: np.asarray(v, dtype=np.float32)
                        for k, v in inputs.items() if k != "x"})
    if n not in _cache:
        _cache[n] = build_nc(n)
    nc = _cache[n]
    in_maps = [{"x": np.ascontiguousarray(x[c]), **w} for c in range(B)]
    res = run_bass_kernel_spmd(nc, in_maps, list(range(NCORES)))
    out = np.stack([res.results[c]["out"][0] for c in range(B)], axis=0)
    return out.astype(np.float32)


# revision 33
# speedup vs baseline: 3.8304x; 1.0033x over previous
"""DGCNN prediction head on 8 Trainium2 NeuronCores.

Data-parallel over batch B=8: each core runs the full pipeline for one
sample (C=64 channels, N=4096 points, k=20 neighbors).

Per-core pipeline (all on one NeuronCore, no collectives):
  1. pairwise ranking R[i,j] = 2<x_i,x_j> - ||x_j||^2 via PE matmul with an
     augmented contract row; R stays in PSUM (no drain).
  2. top-20 per row, chunked: per 512-col chunk DVE max8 + max_index read
     the PSUM tile directly (2 passes over the row total instead of 8).
     The 64 chunk candidates are packed (value mantissa | column index) so
     a 3-round max8/match_replace on the 64-wide union yields the top-20
     indices with no further full-row scans.
  3. EdgeConv1 is linear before the LReLU, so it is precomputed per point:
       conv1(i,j) = Wn x_j + (Wc - Wn) x_i  with BN1 folded in
     A' = s1*(Wn x)        -> fp16 DRAM table, 20 rows gathered per block
                              in ONE indirect DMA
     B' = s1*((Wc-Wn) x)+t1-> fp16 on-chip, broadcast-added per query block
  4. e1 = lrelu(A'_j + B'_i): add on DVE (fp16 2x), lrelu on GPSIMD;
     PE-transpose to channel-major (fp16); EdgeConv2 as fp16 64x64 matmuls
     (BN2 scale folded into W2, bias t2 added during the PSUM drain);
     max over k as fp16 DVE tree; lrelu (monotone, s2 >= 0).
  5. point MLP 64->256->128->1, fp16 weights, BN scales folded, biases
     added during PSUM drains, lrelu on GPSIMD.
"""

import numpy as np

C = 64
K = 20
NEG = 0.2
EPS = 1e-5
NCORES = 8
N_FULL = 4096
NEG_FILL = -3.0e38
NCH = 4          # top-k chunks per 128-row block
CHW = 1024       # chunk width

_cache = {}


def build_nc(n):
    from contextlib import ExitStack

    import concourse.bass as bass
    import concourse.bacc as bacc
    import concourse.mybir as mybir
    import concourse.tile as tile
    from concourse.masks import make_identity

    f32 = mybir.dt.float32
    f32r = mybir.dt.float32r
    f16 = mybir.dt.float16
    u32 = mybir.dt.uint32
    AF = mybir.ActivationFunctionType
    OP = mybir.AluOpType

    nblk = n // 128
    nchk = n // 512

    nc = bacc.Bacc("TRN2", target_bir_lowering=False, debug=False,
                   num_devices=NCORES)

    x_d = nc.dram_tensor("x", [C, n], f32, kind="ExternalInput")
    wnT_d = nc.dram_tensor("wnT", [C, C], f32, kind="ExternalInput")
    wcnT_d = nc.dram_tensor("wcnT", [C, C], f32, kind="ExternalInput")
    t1_d = nc.dram_tensor("t1", [C, 1], f32, kind="ExternalInput")
    w2T_d = nc.dram_tensor("w2T", [C, C], f16, kind="ExternalInput")
    t2_d = nc.dram_tensor("t2", [C, 1], f32, kind="ExternalInput")
    w1aT_d = nc.dram_tensor("w1aT", [C, 128], f16, kind="ExternalInput")
    w1bT_d = nc.dram_tensor("w1bT", [C, 128], f16, kind="ExternalInput")
    tm1a_d = nc.dram_tensor("tm1a", [128, 1], f32, kind="ExternalInput")
    tm1b_d = nc.dram_tensor("tm1b", [128, 1], f32, kind="ExternalInput")
    w2maT_d = nc.dram_tensor("w2maT", [128, 128], f16, kind="ExternalInput")
    w2mbT_d = nc.dram_tensor("w2mbT", [128, 128], f16, kind="ExternalInput")
    tm2_d = nc.dram_tensor("tm2", [128, 1], f32, kind="ExternalInput")
    w3T_d = nc.dram_tensor("w3T", [128, 1], f16, kind="ExternalInput")
    b3_d = nc.dram_tensor("b3", [1, 1], f32, kind="ExternalInput")
    choff_d = nc.dram_tensor("choff", [128, NCH * 8], u32, kind="ExternalInput")
    out_d = nc.dram_tensor("out", [1, n], f32, kind="ExternalOutput")

    with tile.TileContext(nc) as tc, ExitStack() as top:
        cpool = top.enter_context(tc.tile_pool(name="consts", bufs=1))
        dpool = top.enter_context(tc.tile_pool(name="dram", bufs=1, space="DRAM"))
        xpool = top.enter_context(tc.tile_pool(name="xaug", bufs=1))
        hpool = top.enter_context(tc.tile_pool(name="hout", bufs=1))

        # --- x load issued before const loads so SP serves stage-0 ASAP;
        # quarters across queues so chunk 0 lands early ---
        nq = n // 4
        xaug = xpool.tile([C + 1, n], f32, tag="xaug")   # rows 0..63 = x, row 64 = ||x_j||^2
        nc.sync.dma_start(xaug[:C, 0 * nq:1 * nq], x_d[:, 0 * nq:1 * nq])
        nc.scalar.dma_start(xaug[:C, 1 * nq:2 * nq], x_d[:, 1 * nq:2 * nq])
        nc.gpsimd.dma_start(xaug[:C, 2 * nq:3 * nq], x_d[:, 2 * nq:3 * nq])
        nc.sync.dma_start(xaug[:C, 3 * nq:4 * nq], x_d[:, 3 * nq:4 * nq])

        # --- constants / weights ---
        ident = cpool.tile([128, 128], f32, tag="ident")
        make_identity(nc, ident[:])
        ident16 = cpool.tile([128, 128], f16, tag="ident16")
        nc.scalar.copy(out=ident16[:], in_=ident[:])
        ones64 = cpool.tile([C, 1], f32, tag="ones64")
        nc.vector.memset(ones64[:], 1.0)

        def load_const(dram, shape, tag, dt=f32):
            t = cpool.tile(shape, dt, tag=tag)
            nc.sync.dma_start(t[:], dram[:])
            return t

        wnT = load_const(wnT_d, [C, C], "wnT")
        wcnT = load_const(wcnT_d, [C, C], "wcnT")
        t1 = load_const(t1_d, [C, 1], "t1")
        w2T = load_const(w2T_d, [C, C], "w2T", f16)
        t2 = load_const(t2_d, [C, 1], "t2")
        w1aT = load_const(w1aT_d, [C, 128], "w1aT", f16)
        w1bT = load_const(w1bT_d, [C, 128], "w1bT", f16)
        tm1a = load_const(tm1a_d, [128, 1], "tm1a")
        tm1b = load_const(tm1b_d, [128, 1], "tm1b")
        w2maT = load_const(w2maT_d, [128, 128], "w2maT", f16)
        w2mbT = load_const(w2mbT_d, [128, 128], "w2mbT", f16)
        tm2 = load_const(tm2_d, [128, 1], "tm2")
        w3T = load_const(w3T_d, [128, 1], "w3T", f16)
        b3 = load_const(b3_d, [1, 1], "b3")
        choff = load_const(choff_d, [128, NCH * 8], "choff", u32)

        At = dpool.tile([n, C], f16, tag="At")           # A' fp16 gather table
        x2aug = xpool.tile([C + 1, n], f32, tag="x2aug") # rows 0..63 = 2x, row 64 = -1
        Bt = xpool.tile([128, C * nblk], f16, tag="Bt")  # B' fp16, block j at cols 64j
        H = hpool.tile([C, n], f16, tag="H")             # per-point features after edge max
        osb = hpool.tile([1, n], f32, tag="osb")

        # ---------------- stage 0: tables ----------------
        with tc.tile_pool(name="s0sb", bufs=2) as s0sb, \
             tc.tile_pool(name="s0ps", bufs=3, space="PSUM") as s0ps, \
             tc.tile_pool(name="s0pt", bufs=2, space="PSUM") as s0pt:
            nc.gpsimd.memset(x2aug[C:C + 1, :], -1.0)
            for ch in range(nchk):
                cs = slice(512 * ch, 512 * (ch + 1))
                # 2x copy + squares per chunk (Act), drains split
                nc.scalar.activation(out=x2aug[:C, cs], in_=xaug[:C, cs],
                                     func=AF.Copy, scale=2.0)
                xsq = s0sb.tile([C, 512], f32, tag="xsq")
                nc.scalar.activation(out=xsq[:], in_=xaug[:C, cs],
                                     func=AF.Square)
                psxx = s0ps.tile([1, 512], f32, tag="s0p", space="PSUM")
                nc.tensor.matmul(out=psxx[:], lhsT=ones64[:].bitcast(f32r),
                                 rhs=xsq[:].bitcast(f32r),
                                 start=True, stop=True)
                nc.gpsimd.tensor_copy(out=xaug[C:C + 1, cs], in_=psxx[:])
                psa = s0ps.tile([C, 512], f32, tag="s0p", space="PSUM")
                nc.tensor.matmul(out=psa[:], lhsT=wnT[:].bitcast(f32r),
                                 rhs=xaug[:C, cs].bitcast(f32r),
                                 start=True, stop=True)
                ap = s0sb.tile([C, 512], f16, tag="ap")
                nc.gpsimd.tensor_copy(out=ap[:], in_=psa[:])
                psb = s0ps.tile([C, 512], f32, tag="s0p", space="PSUM")
                nc.tensor.matmul(out=psb[:], lhsT=wcnT[:].bitcast(f32r),
                                 rhs=xaug[:C, cs].bitcast(f32r),
                                 start=True, stop=True)
                bp = s0sb.tile([C, 512], f16, tag="bp")
                nc.vector.tensor_scalar(out=bp[:], in0=psb[:], scalar1=t1[:],
                                        scalar2=None, op0=OP.add)
                ast = s0sb.tile([128, 4 * C], f16, tag="ast")
                for j in range(4):
                    blk = 4 * ch + j
                    js = slice(128 * j, 128 * (j + 1))
                    pta = s0pt.tile([128, C], f16, tag="s0t", space="PSUM")
                    nc.tensor.transpose(out=pta[:], in_=ap[:, js],
                                        identity=ident16[:C, :C])
                    nc.gpsimd.tensor_copy(out=ast[:, C * j:C * (j + 1)],
                                          in_=pta[:])
                    ptb = s0pt.tile([128, C], f16, tag="s0t", space="PSUM")
                    nc.tensor.transpose(out=ptb[:], in_=bp[:, js],
                                        identity=ident16[:C, :C])
                    nc.scalar.copy(out=Bt[:, C * blk:C * (blk + 1)], in_=ptb[:])
                nc.sync.dma_start(
                    At[512 * ch:512 * (ch + 1), :].rearrange(
                        "(j p) c -> p j c", p=128),
                    ast[:].rearrange("p (j c) -> p j c", j=4))

        # ---------------- stage 1: blocks ----------------
        with tc.tile_pool(name="cpoolv", bufs=2) as cvp, \
             tc.tile_pool(name="gpool", bufs=2) as gpool, \
             tc.tile_pool(name="epool", bufs=2) as epool, \
             tc.tile_pool(name="wpool", bufs=2) as wpool, \
             tc.tile_pool(name="tpool", bufs=2) as tpool, \
             tc.tile_pool(name="mlpsb", bufs=2) as mlpsb, \
             tc.tile_pool(name="psR", bufs=2, space="PSUM") as psR, \
             tc.tile_pool(name="psT", bufs=1, space="PSUM") as psT, \
             tc.tile_pool(name="psE", bufs=2, space="PSUM") as psE, \
             tc.tile_pool(name="mlpps", bufs=1, space="PSUM") as mlpps:

            def mlp_chunk(ch):
                cs = slice(512 * ch, 512 * (ch + 1))
                l1a = mlpsb.tile([128, 512], f16, tag="l1a")
                l1b = mlpsb.tile([128, 512], f16, tag="l1b")
                ps1a = mlpps.tile([128, 512], f32, tag="mlpp", space="PSUM")
                nc.tensor.matmul(out=ps1a[:], lhsT=w1aT[:], rhs=H[:, cs],
                                 start=True, stop=True)
                nc.scalar.activation(out=l1a[:], in_=ps1a[:],
                                     func=AF.Identity, bias=tm1a[:], scale=1.0)
                nc.gpsimd.scalar_tensor_tensor(
                    out=l1a[:], in0=l1a[:], scalar=NEG, in1=l1a[:],
                    op0=OP.mult, op1=OP.max)
                ps1b = mlpps.tile([128, 512], f32, tag="mlpp", space="PSUM")
                nc.tensor.matmul(out=ps1b[:], lhsT=w1bT[:], rhs=H[:, cs],
                                 start=True, stop=True)
                nc.scalar.activation(out=l1b[:], in_=ps1b[:],
                                     func=AF.Identity, bias=tm1b[:], scale=1.0)
                nc.gpsimd.scalar_tensor_tensor(
                    out=l1b[:], in0=l1b[:], scalar=NEG, in1=l1b[:],
                    op0=OP.mult, op1=OP.max)
                ps2 = mlpps.tile([128, 512], f32, tag="mlpp", space="PSUM")
                nc.tensor.matmul(out=ps2[:], lhsT=w2maT[:], rhs=l1a[:],
                                 start=True, stop=False)
                nc.tensor.matmul(out=ps2[:], lhsT=w2mbT[:], rhs=l1b[:],
                                 start=False, stop=True)
                l2 = mlpsb.tile([128, 512], f16, tag="l2")
                nc.scalar.activation(out=l2[:], in_=ps2[:],
                                     func=AF.Identity, bias=tm2[:], scale=1.0)
                nc.gpsimd.scalar_tensor_tensor(
                    out=l2[:], in0=l2[:], scalar=NEG, in1=l2[:],
                    op0=OP.mult, op1=OP.max)
                ps3 = mlpps.tile([1, 512], f32, tag="mlpp", space="PSUM")
                nc.tensor.matmul(out=ps3[:], lhsT=w3T[:], rhs=l2[:],
                                 start=True, stop=True)
                nc.scalar.activation(out=osb[:, cs], in_=ps3[:],
                                     func=AF.Identity, bias=b3[:], scale=1.0)

            for b in range(nblk):
                bs = slice(128 * b, 128 * (b + 1))

                # pairwise + chunked top-8, straight from PSUM
                cand_v = cvp.tile([128, NCH * 8], f32, tag="cv")
                cand_i = cvp.tile([128, NCH * 8], u32, tag="ci")
                for ch in range(NCH):
                    ks = slice(8 * ch, 8 * (ch + 1))
                    ps = psR.tile([128, CHW], f32, tag="psr", space="PSUM")
                    for hh in range(CHW // 512):
                        c0 = CHW * ch + 512 * hh
                        nc.tensor.matmul(
                            out=ps[:, 512 * hh:512 * (hh + 1)],
                            lhsT=x2aug[:, bs].bitcast(f32r),
                            rhs=xaug[:, c0:c0 + 512].bitcast(f32r),
                            start=True, stop=True)
                    nc.vector.max(out=cand_v[:, ks], in_=ps[:])
                    nc.vector.max_index(out=cand_i[:, ks],
                                        in_max=cand_v[:, ks], in_values=ps[:])

                # pack value|index on GPSIMD, union top-20 on DVE
                pk = cvp.tile([128, NCH * 8], u32, tag="pk")
                nc.gpsimd.tensor_tensor(out=cand_i[:], in0=cand_i[:],
                                        in1=choff[:], op=OP.add)
                nc.gpsimd.tensor_scalar(out=pk[:], in0=cand_v[:].bitcast(u32),
                                        scalar1=0xFFFFF000, scalar2=None,
                                        op0=OP.bitwise_and)
                nc.gpsimd.tensor_tensor(out=pk[:], in0=pk[:], in1=cand_i[:],
                                        op=OP.bitwise_or)
                pkf = pk[:].bitcast(f32)
                pv1 = cvp.tile([128, 8], f32, tag="pv1")
                pv2 = cvp.tile([128, 8], f32, tag="pv2")
                pv3 = cvp.tile([128, 8], f32, tag="pv3")
                nc.vector.max(out=pv1[:], in_=pkf)
                nc.vector.match_replace(out=pkf, in_to_replace=pv1[:],
                                        in_values=pkf, imm_value=NEG_FILL)
                nc.vector.max(out=pv2[:], in_=pkf)
                nc.vector.match_replace(out=pkf, in_to_replace=pv2[:],
                                        in_values=pkf, imm_value=NEG_FILL)
                nc.vector.max(out=pv3[:], in_=pkf)
                idx = cvp.tile([128, 24], u32, tag="idx")
                nc.gpsimd.tensor_scalar(out=idx[:, 0:8], in0=pv1[:].bitcast(u32),
                                        scalar1=0xFFF, scalar2=None,
                                        op0=OP.bitwise_and)
                nc.gpsimd.tensor_scalar(out=idx[:, 8:16], in0=pv2[:].bitcast(u32),
                                        scalar1=0xFFF, scalar2=None,
                                        op0=OP.bitwise_and)
                nc.gpsimd.tensor_scalar(out=idx[:, 16:24], in0=pv3[:].bitcast(u32),
                                        scalar1=0xFFF, scalar2=None,
                                        op0=OP.bitwise_and)

                # gather all 20 neighbors in one indirect DMA (fp16 rows)
                G = gpool.tile([128, K * C], f16, tag="G")
                nc.gpsimd.indirect_dma_start(
                    out=G[:].rearrange("p (k c) -> p k c", k=K),
                    out_offset=None,
                    in_=At[:],
                    in_offset=bass.IndirectOffsetOnAxis(ap=idx[:, 0:K], axis=0))

                # e1 = lrelu(G + B'_i): add + lrelu both on GPSIMD
                bb = Bt[:, C * b:C * (b + 1)].rearrange(
                    "p (k c) -> p k c", k=1).to_broadcast([128, K, C])
                nc.gpsimd.tensor_tensor(
                    out=G[:].rearrange("p (k c) -> p k c", k=K),
                    in0=G[:].rearrange("p (k c) -> p k c", k=K),
                    in1=bb, op=OP.add)
                nc.gpsimd.scalar_tensor_tensor(
                    out=G[:], in0=G[:], scalar=NEG, in1=G[:],
                    op0=OP.mult, op1=OP.max)

                # transpose to channel-major: 20 PE transposes [128,64]->[64,128]
                e1T = epool.tile([C, K * 128], f16, tag="e1T")
                for grp, gw in ((0, 8), (1, 8), (2, 4)):
                    pt = psT.tile([C, 1024], f16, tag="pst", space="PSUM")
                    for s in range(gw):
                        k = 8 * grp + s
                        nc.tensor.transpose(
                            out=pt[:, 128 * s:128 * (s + 1)],
                            in_=G[:, C * k:C * (k + 1)],
                            identity=ident16[:])
                    nc.scalar.copy(
                        out=e1T[:, 1024 * grp:1024 * grp + 128 * gw],
                        in_=pt[:, :128 * gw])

                # conv2 (w_k2 with bn2 scale folded), t2 added in drain
                ew = wpool.tile([C, K * 128], f16, tag="ew")
                for grp in range(5):
                    pe = psE.tile([C, 512], f32, tag="pse", space="PSUM")
                    for s in range(4):
                        k = 4 * grp + s
                        nc.tensor.matmul(
                            out=pe[:, 128 * s:128 * (s + 1)],
                            lhsT=w2T[:],
                            rhs=e1T[:, 128 * k:128 * (k + 1)],
                            start=True, stop=True)
                    nc.scalar.activation(
                        out=ew[:, 512 * grp:512 * (grp + 1)], in_=pe[:],
                        func=AF.Identity, bias=t2[:], scale=1.0)

                # max over k: levels 1-2 on GPSIMD, rest + lrelu on DVE
                m1 = tpool.tile([C, 10 * 128], f16, tag="m1")
                nc.gpsimd.tensor_tensor(out=m1[:], in0=ew[:, :1280],
                                        in1=ew[:, 1280:], op=OP.max)
                m2 = tpool.tile([C, 5 * 128], f16, tag="m2")
                nc.gpsimd.tensor_tensor(out=m2[:], in0=m1[:, :640],
                                        in1=m1[:, 640:], op=OP.max)
                m3 = tpool.tile([C, 2 * 128], f16, tag="m3")
                nc.gpsimd.tensor_tensor(out=m3[:], in0=m2[:, :256],
                                        in1=m2[:, 256:512], op=OP.max)
                m4 = tpool.tile([C, 128], f16, tag="m4")
                nc.gpsimd.tensor_tensor(out=m4[:], in0=m3[:, :128],
                                        in1=m3[:, 128:], op=OP.max)
                nc.gpsimd.tensor_tensor(out=m4[:], in0=m4[:],
                                        in1=m2[:, 512:], op=OP.max)
                nc.gpsimd.scalar_tensor_tensor(
                    out=H[:, bs], in0=m4[:], scalar=NEG, in1=m4[:],
                    op0=OP.mult, op1=OP.max)

                # point MLP for the finished 512-col chunk, interleaved
                if b % 4 == 3:
                    mlp_chunk(b // 4)

            nc.sync.dma_start(out_d[:], osb[:])

    nc.finalize()
    return nc


def host_weights(w_k1, g_k1, b_k1, m_k1, v_k1, w_k2, g_k2, b_k2, m_k2, v_k2,
                 w1, g1, b1, m1, v1, w2, g2, b2, m2, v2, w3, b3):
    f = np.float32
    h = np.float16
    s1 = (g_k1 / np.sqrt(v_k1 + f(EPS))).astype(f)
    t1 = (b_k1 - m_k1 * s1).astype(f)
    wn = w_k1[:, :C]
    wc = w_k1[:, C:]
    wnT = np.ascontiguousarray((wn * s1[:, None]).T.astype(f))
    wcnT = np.ascontiguousarray(((wc - wn) * s1[:, None]).T.astype(f))
    s2 = (g_k2 / np.sqrt(v_k2 + f(EPS))).astype(f)
    t2 = (b_k2 - m_k2 * s2).astype(f)
    w2T = np.ascontiguousarray((w_k2 * s2[:, None]).T.astype(h))
    sm1 = (g1 / np.sqrt(v1 + f(EPS))).astype(f)
    tm1 = (b1 - m1 * sm1).astype(f)
    w1s = (w1 * sm1[:, None]).astype(f)            # (256, 64)
    w1aT = np.ascontiguousarray(w1s[:128].T.astype(h))  # (64, 128)
    w1bT = np.ascontiguousarray(w1s[128:].T.astype(h))
    sm2 = (g2 / np.sqrt(v2 + f(EPS))).astype(f)
    tm2 = (b2 - m2 * sm2).astype(f)
    w2s = (w2 * sm2[:, None]).astype(f)            # (128, 256)
    w2maT = np.ascontiguousarray(w2s[:, :128].T.astype(h))  # (128, 128)
    w2mbT = np.ascontiguousarray(w2s[:, 128:].T.astype(h))
    w3T = np.ascontiguousarray(w3.T.astype(h))     # (128, 1)
    choff = np.broadcast_to(
        (np.repeat(np.arange(NCH, dtype=np.uint32), 8) * CHW)[None, :],
        (128, NCH * 8))
    return {
        "wnT": wnT, "wcnT": wcnT, "t1": t1.reshape(C, 1),
        "w2T": w2T, "t2": t2.reshape(C, 1),
        "w1aT": w1aT, "w1bT": w1bT,
        "tm1a": tm1[:128].reshape(128, 1), "tm1b": tm1[128:].reshape(128, 1),
        "w2maT": w2maT, "w2mbT": w2mbT, "tm2": tm2.reshape(128, 1),
        "w3T": w3T, "b3": b3.reshape(1, 1).astype(f),
        "choff": np.ascontiguousarray(choff),
    }


def kernel(**inputs):
    from concourse.bass_utils import run_bass_kernel_spmd

    x = np.asarray(inputs["x"], dtype=np.float32)  # (B, C, N)
    B = x.shape[0]
    n = x.shape[2]
    w = host_weights(**{k: np.asarray(v, dtype=np.float32)
                        for k, v in inputs.items() if k != "x"})
    if n not in _cache:
        _cache[n] = build_nc(n)
    nc = _cache[n]
    in_maps = [{"x": np.ascontiguousarray(x[c]), **w} for c in range(B)]
    res = run_bass_kernel_spmd(nc, in_maps, list(range(NCORES)))
    out = np.stack([res.results[c]["out"][0] for c in range(B)], axis=0)
    return out.astype(np.float32)
